# revision 2
# baseline (speedup 1.0000x reference)
"""ClassAttention kernel for 8x TRN2 NeuronCores.

Reference computation (per batch element):
    qkv = x @ qkv_w.T + qkv_b                      # [N, 3C]
    q, k, v = split(qkv)                           # heads H=12, D=64
    s = softmax((q_cls . k) / sqrt(D))             # class-token query only
    cls = (s @ v) @ proj_w.T + proj_b              # [1, C]
    out = concat([cls, x[1:]])                     # rows 1..N pass through

Only the class token row changes, so the device kernel computes just the
[B, C] cls output; rows 1..N are passed through on the host.

Sharding: data-parallel over batch, 8 batches per core, no collectives.
Compute dtype bf16 (fp32 PSUM accumulation), storage/IO fp32 for biases
and the output.

Per-core dataflow (b = 0..8 batches, C=768 in 6 chunks of 128):
  xT[c, b, n]  (host-transposed, bf16)
  kT_b[o, n]  = wkvT[:, :768].T @ xT_b   (+k bias)     o-major for scores
  v_b[n, o]   = xT_b.T @ wkvT[:, 768:]   (+v bias)     n-major for AV
  qc[b, o]    = wqT.T @ xcls (+q bias)   (pre-scaled by 1/8 on host)
  s_b[h, n]   = blockdiag(q_b).T @ kT_b                one matmul chain/half
  p_b         = exp(s - max)             (unnormalized; 1/sum folded later)
  o_b[h, o]   = pT_b.T @ v_b             (diag blocks extracted via PE
                                          transpose + aligned copies)
  cls[b, :]   = oT.T @ wpT + proj_b
"""

import functools

import numpy as np
import ml_dtypes

import concourse.bass as bass
import concourse.tile as tile
from concourse import bacc, mybir, masks
from concourse import bass_utils

BF16 = mybir.dt.bfloat16
F32 = mybir.dt.float32
NPBF16 = ml_dtypes.bfloat16

B, N, C = 64, 577, 768
H, D = 12, 64
NCORES = 8
BPC = B // NCORES          # 8 batches per core
CT = C // 128              # 6 chunks of the feature dim
SCALE = D ** -0.5          # folded into wq/qb on the host

# token splits: matmul free dim (<=512 fp32 psum bank), K-chunks (<=128)
N_HALVES = [(0, 289), (289, 288)]
C_HALVES = [(0, 512), (512, 256)]
T_TILES = [(0, 128), (128, 128), (256, 128), (384, 128), (512, 65)]
NTT = len(T_TILES)


def build_module():
    nc = bacc.Bacc("TRN2", target_bir_lowering=False, debug=False)

    xT_d = nc.dram_tensor("xT", [C, BPC, N], BF16, kind="ExternalInput")
    wkv_d = nc.dram_tensor("wkv", [C, 2 * C], BF16, kind="ExternalInput")
    wq_d = nc.dram_tensor("wq", [C, C], BF16, kind="ExternalInput")
    wp_d = nc.dram_tensor("wp", [C, C], BF16, kind="ExternalInput")
    xcls_d = nc.dram_tensor("xcls", [C, BPC], BF16, kind="ExternalInput")
    kvb_d = nc.dram_tensor("kvb", [2 * C], F32, kind="ExternalInput")
    qb_d = nc.dram_tensor("qb", [BPC, C], F32, kind="ExternalInput")
    pb_d = nc.dram_tensor("pb", [BPC, C], F32, kind="ExternalInput")
    vb_d = nc.dram_tensor("vb", [128, C], F32, kind="ExternalInput")
    cls_d = nc.dram_tensor("cls", [BPC, C], F32, kind="ExternalOutput")

    AF = mybir.ActivationFunctionType
    AX = mybir.AxisListType

    with tile.TileContext(nc) as tc:
        with (
            tc.tile_pool(name="const", bufs=1) as const,
            tc.tile_pool(name="xp", bufs=2) as xp,
            tc.tile_pool(name="kv", bufs=2) as kvp,
            tc.tile_pool(name="sm", bufs=2) as sm,
            tc.tile_pool(name="ps", bufs=2, space="PSUM") as ps,
        ):
            # ---- resident constants ----
            wkv = const.tile([128, CT, 2 * C], BF16, tag="wkv")
            nc.sync.dma_start(
                wkv[:], wkv_d.ap().rearrange("(a p) o -> p a o", p=128))
            wq = const.tile([128, CT, C], BF16, tag="wq")
            nc.sync.dma_start(
                wq[:], wq_d.ap().rearrange("(a p) o -> p a o", p=128))
            wp = const.tile([128, CT, C], BF16, tag="wp")
            nc.sync.dma_start(
                wp[:], wp_d.ap().rearrange("(a p) o -> p a o", p=128))
            xcls = const.tile([128, CT, BPC], BF16, tag="xcls")
            nc.sync.dma_start(
                xcls[:], xcls_d.ap().rearrange("(a p) b -> p a b", p=128))
            kvbT = const.tile([128, 2 * CT], F32, tag="kvbT")
            nc.sync.dma_start(
                kvbT[:], kvb_d.ap().rearrange("(a p) -> p a", p=128))
            vbr = const.tile([128, C], F32, tag="vbr")
            nc.sync.dma_start(vbr[:], vb_d.ap())
            qbr = const.tile([BPC, C], F32, tag="qbr")
            nc.sync.dma_start(qbr[:], qb_d.ap())
            pbr = const.tile([BPC, C], F32, tag="pbr")
            nc.sync.dma_start(pbr[:], pb_d.ap())

            identb = const.tile([12, 12], BF16, tag="identb")
            masks.make_identity(nc, identb[:])
            identf = const.tile([BPC, BPC], F32, tag="identf")
            masks.make_identity(nc, identf[:])

            Qblk = const.tile([128, CT, BPC * H], BF16, tag="Qblk")
            nc.vector.memset(Qblk[:], 0.0)
            oT_vec = const.tile([128, CT, BPC], BF16, tag="oT_vec")
            q_sb = const.tile([BPC, C], F32, tag="q_sb")

            # ---- stage B: class-token q for all 8 batches ----
            for ho, hw in C_HALVES:
                pq = ps.tile([BPC, 512], F32, tag="pbh")
                for ci in range(CT):
                    nc.tensor.matmul(
                        pq[:, :hw], xcls[:, ci, :], wq[:, ci, ho:ho + hw],
                        start=(ci == 0), stop=(ci == CT - 1))
                nc.vector.tensor_add(
                    q_sb[:, ho:ho + hw], pq[:, :hw], qbr[:, ho:ho + hw])

            # build block-diagonal Q: Qblk[o, b*12+h] = q_sb[b, o], o in head h
            QblkV = Qblk[:].rearrange("p a (b h) -> p a b h", h=H)
            for ci in range(CT):
                pqt = ps.tile([128, BPC], F32, tag="ptrf")
                nc.tensor.transpose(
                    pqt[:, :], q_sb[:, ci * 128:(ci + 1) * 128],
                    identf[:BPC, :BPC])
                for j in range(2):
                    h = 2 * ci + j
                    nc.vector.tensor_copy(
                        QblkV[j * 64:(j + 1) * 64, ci, :, h],
                        pqt[j * 64:(j + 1) * 64, :])

            # ---- per-batch pipeline ----
            for b in range(BPC):
                xb = xp.tile([128, CT, N], BF16, tag="xb")
                nc.sync.dma_start(
                    xb[:],
                    xT_d.ap()[:, b, :].rearrange("(a p) t -> p a t", p=128))

                # C1: kT_b[o, t] (+ bias per-partition), o-major
                kT = kvp.tile([128, CT, N], BF16, tag="kT")
                for oi in range(CT):
                    for no, nw in N_HALVES:
                        pk = ps.tile([128, 512], F32, tag="pkv")
                        for ci in range(CT):
                            nc.tensor.matmul(
                                pk[:, :nw],
                                wkv[:, ci, oi * 128:(oi + 1) * 128],
                                xb[:, ci, no:no + nw],
                                start=(ci == 0), stop=(ci == CT - 1))
                        nc.scalar.add(
                            kT[:, oi, no:no + nw], pk[:, :nw],
                            kvbT[:, oi:oi + 1])

                # C2: v_b[t, o] (+ bias along free), t-major
                vt = kvp.tile([128, NTT, C], BF16, tag="vt")
                for ti, (to, tw) in enumerate(T_TILES):
                    for ho, hw in C_HALVES:
                        pv = ps.tile([128, 512], F32, tag="pkv")
                        for ci in range(CT):
                            nc.tensor.matmul(
                                pv[:tw, :hw],
                                xb[:, ci, to:to + tw],
                                wkv[:, ci, C + ho:C + ho + hw],
                                start=(ci == 0), stop=(ci == CT - 1))
                        nc.vector.tensor_add(
                            vt[:tw, ti, ho:ho + hw], pv[:tw, :hw],
                            vbr[:tw, ho:ho + hw])

                # C3 + C4: scores + softmax (p left unnormalized)
                pss = []
                for no, nw in N_HALVES:
                    s_ps = ps.tile([H, 512], F32, tag="pbh")
                    for ci in range(CT):
                        nc.tensor.matmul(
                            s_ps[:, :nw],
                            Qblk[:, ci, b * H:(b + 1) * H],
                            kT[:, ci, no:no + nw],
                            start=(ci == 0), stop=(ci == CT - 1))
                    pss.append(s_ps)

                m1 = sm.tile([H, 1], F32, tag="m1")
                m2 = sm.tile([H, 1], F32, tag="m2")
                nc.vector.reduce_max(m1[:], pss[0][:, :N_HALVES[0][1]], axis=AX.X)
                nc.vector.reduce_max(m2[:], pss[1][:, :N_HALVES[1][1]], axis=AX.X)
                am = sm.tile([H, 1], F32, tag="am")
                nc.vector.tensor_max(am[:], m1[:], m2[:])
                negm = sm.tile([H, 1], F32, tag="negm")
                nc.scalar.mul(negm[:], am[:], -1.0)

                e_bf = sm.tile([H, N], BF16, tag="e_bf")
                sums = []
                for i, (no, nw) in enumerate(N_HALVES):
                    acc = sm.tile([H, 1], F32, tag=f"acc{i}")
                    nc.scalar.activation(
                        e_bf[:, no:no + nw], pss[i][:, :nw], AF.Exp,
                        bias=negm[:], scale=1.0, accum_out=acc[:])
                    sums.append(acc)
                ssum = sm.tile([H, 1], F32, tag="ssum")
                nc.vector.tensor_add(ssum[:], sums[0][:], sums[1][:])
                rden = sm.tile([H, 1], F32, tag="rden")
                nc.vector.reciprocal(rden[:], ssum[:])

                # C5: transpose p -> pT[t, h]
                pT = sm.tile([128, NTT, H], BF16, tag="pT")
                for ti, (to, tw) in enumerate(T_TILES):
                    ppt = ps.tile([128, H], BF16, tag="ptrb")
                    nc.tensor.transpose(
                        ppt[:tw, :], e_bf[:, to:to + tw], identb[:H, :H])
                    nc.vector.tensor_copy(pT[:tw, ti, :], ppt[:tw, :])

                # C6: o_b[h, o] = pT.T @ v  (full 12x768, diag blocks wanted)
                o_tmp = sm.tile([H, C], BF16, tag="o_tmp")
                for ho, hw in C_HALVES:
                    po = ps.tile([H, 512], F32, tag="pbh")
                    for ti, (to, tw) in enumerate(T_TILES):
                        nc.tensor.matmul(
                            po[:, :hw],
                            pT[:tw, ti, :],
                            vt[:tw, ti, ho:ho + hw],
                            start=(ti == 0), stop=(ti == NTT - 1))
                    # normalize by 1/sum while evacuating psum
                    nc.vector.tensor_scalar_mul(
                        o_tmp[:, ho:ho + hw], po[:, :hw], rden[:])

                # C7: extract diagonal blocks, transposed: oT_vec[o, b]
                for ci in range(CT):
                    pot = ps.tile([128, H], BF16, tag="ptrb")
                    nc.tensor.transpose(
                        pot[:, :], o_tmp[:, ci * 128:(ci + 1) * 128],
                        identb[:H, :H])
                    for j in range(2):
                        h = 2 * ci + j
                        nc.vector.tensor_copy(
                            oT_vec[j * 64:(j + 1) * 64, ci, b:b + 1],
                            pot[j * 64:(j + 1) * 64, h:h + 1])

            # ---- C8: proj for all 8 batches ----
            cls_sb = const.tile([BPC, C], F32, tag="cls_sb")
            for ho, hw in C_HALVES:
                pc = ps.tile([BPC, 512], F32, tag="pbh")
                for ci in range(CT):
                    nc.tensor.matmul(
                        pc[:, :hw], oT_vec[:, ci, :], wp[:, ci, ho:ho + hw],
                        start=(ci == 0), stop=(ci == CT - 1))
                nc.vector.tensor_add(
                    cls_sb[:, ho:ho + hw], pc[:, :hw], pbr[:, ho:ho + hw])
            nc.sync.dma_start(cls_d.ap(), cls_sb[:])

    nc.compile()
    return nc


@functools.lru_cache(maxsize=1)
def _module():
    return build_module()


def make_in_maps(x, qkv_w, qkv_b, proj_w, proj_b):
    x = np.asarray(x, dtype=np.float32)
    qkv_w = np.asarray(qkv_w, dtype=np.float32)
    qkv_b = np.asarray(qkv_b, dtype=np.float32)
    proj_w = np.asarray(proj_w, dtype=np.float32)
    proj_b = np.asarray(proj_b, dtype=np.float32)

    wkv = np.ascontiguousarray(qkv_w[C:].T).astype(NPBF16)          # [C, 2C]
    wq = np.ascontiguousarray(qkv_w[:C].T * SCALE).astype(NPBF16)   # [C, C]
    wp = np.ascontiguousarray(proj_w.T).astype(NPBF16)              # [C, C]
    kvb = np.ascontiguousarray(qkv_b[C:])                           # [2C]
    qb = np.tile(qkv_b[:C] * SCALE, (BPC, 1)).astype(np.float32)    # [8, C]
    pb = np.tile(proj_b, (BPC, 1)).astype(np.float32)               # [8, C]
    vb = np.tile(qkv_b[2 * C:], (128, 1)).astype(np.float32)        # [128, C]

    in_maps = []
    for i in range(NCORES):
        xs = x[i * BPC:(i + 1) * BPC]                               # [8, N, C]
        xT = np.ascontiguousarray(xs.transpose(2, 0, 1)).astype(NPBF16)
        xcls = np.ascontiguousarray(xs[:, 0, :].T).astype(NPBF16)   # [C, 8]
        in_maps.append({
            "xT": xT, "wkv": wkv, "wq": wq, "wp": wp, "xcls": xcls,
            "kvb": kvb, "qb": qb, "pb": pb, "vb": vb,
        })
    return in_maps


def kernel(x, qkv_w, qkv_b, proj_w, proj_b):
    nc = _module()
    in_maps = make_in_maps(x, qkv_w, qkv_b, proj_w, proj_b)
    res = bass_utils.run_bass_kernel_spmd(
        nc, in_maps, core_ids=list(range(NCORES)))
    out = np.array(np.asarray(x), dtype=np.float32, copy=True)
    for i in range(NCORES):
        out[i * BPC:(i + 1) * BPC, 0, :] = res.results[i]["cls"]
    return out


# revision 4
# speedup vs baseline: 1.4745x; 1.4745x over previous
"""ClassAttention kernel for 8x TRN2 NeuronCores.

Reference computation (per batch element):
    qkv = x @ qkv_w.T + qkv_b                      # [N, 3C]
    q, k, v = split(qkv)                           # heads H=12, D=64
    s = softmax((q_cls . k) / sqrt(D))             # class-token query only
    cls = (s @ v) @ proj_w.T + proj_b              # [1, C]
    out = concat([cls, x[1:]])                     # rows 1..N pass through

Only the class token row changes, so the device kernel computes just the
[B, C] cls output; rows 1..N are passed through on the host.

Sharding: data-parallel over batch, 8 batches per core, no collectives.
Compute dtype bf16 (fp32 PSUM accumulation), fp32 output.

Key algebraic restructuring (class-token query only):
  - scores fold the k-projection into a per-batch effective query in
    x-space:  s[b,h,n] = sum_c Wt[c, b*12+h] x[b,n,c]  with
    Wt = wk.T @ blockdiag(q) computed ONCE for all batches (768x96)
    instead of projecting k for every token (eliminates 2/3 of the
    projection FLOPs of a naive k+v implementation).
  - k bias shifts every score of a head by the same constant ->
    cancels in softmax; dropped.
  - v bias contributes sum_n(p)=1 times vb to the attention output ->
    folds into the proj bias on the host: pb_eff = proj_b + vb @ proj_w.T.
  - softmax normalization (1/sum) is applied to the attention OUTPUT
    rows while evacuating the AV psum.

Per-core dataflow (b = 0..8 batches, C=768 in 6 chunks of 128):
  xT[c, b, n]   host-transposed bf16 input
  qc[b, o]    = wqT.T @ xcls + qb    (wq,qb pre-scaled by 1/8 on host)
  Qblk[o, bh] = block-diagonal q     (PE transpose + aligned copies)
  Wt[c, bh]   = wk2.T @ Qblk         (36 matmuls, once)
  v_b[t, o]   = xT_b.T @ wvT         (no bias; psum evacuated on ACT)
  s_b[h, n]   = Wt_b.T @ xT_b
  p_b         = exp(s - max)          (unnormalized, bf16)
  o_b[h, o]   = pT_b.T @ v_b          (12x768; diag blocks are the result)
  oT_vec[o,b] = r * diag-extract      (PE transpose + aligned copies)
  cls[b, :]   = oT.T @ wpT + pb_eff
"""

import functools

import numpy as np
import ml_dtypes

import concourse.bass as bass
import concourse.tile as tile
from concourse import bacc, mybir, masks
from concourse import bass_utils

BF16 = mybir.dt.bfloat16
F32 = mybir.dt.float32
NPBF16 = ml_dtypes.bfloat16

B, N, C = 64, 577, 768
H, D = 12, 64
NCORES = 8
BPC = B // NCORES          # 8 batches per core
CT = C // 128              # 6 chunks of the feature dim
SCALE = D ** -0.5          # folded into wq/qb on the host

# token splits: matmul free dim (<=512 fp32 psum bank), K-chunks (<=128)
N_HALVES = [(0, 289), (289, 288)]
C_HALVES = [(0, 512), (512, 256)]
T_TILES = [(0, 128), (128, 128), (256, 128), (384, 128), (512, 65)]
NTT = len(T_TILES)


def build_module():
    nc = bacc.Bacc("TRN2", target_bir_lowering=False, debug=False)

    xT_d = nc.dram_tensor("xT", [C, BPC, N], BF16, kind="ExternalInput")
    wk2_d = nc.dram_tensor("wk2", [C, C], BF16, kind="ExternalInput")  # [o, c]
    wv_d = nc.dram_tensor("wv", [C, C], BF16, kind="ExternalInput")    # [c, o]
    wq_d = nc.dram_tensor("wq", [C, C], BF16, kind="ExternalInput")    # [c, o]
    wp_d = nc.dram_tensor("wp", [C, C], BF16, kind="ExternalInput")    # [c, o]
    xcls_d = nc.dram_tensor("xcls", [C, BPC], BF16, kind="ExternalInput")
    qb_d = nc.dram_tensor("qb", [BPC, C], F32, kind="ExternalInput")
    pb_d = nc.dram_tensor("pb", [BPC, C], F32, kind="ExternalInput")
    cls_d = nc.dram_tensor("cls", [BPC, C], F32, kind="ExternalOutput")

    AF = mybir.ActivationFunctionType
    AX = mybir.AxisListType

    with tile.TileContext(nc) as tc:
        with (
            tc.tile_pool(name="const", bufs=1) as const,
            tc.tile_pool(name="xp", bufs=2) as xp,
            tc.tile_pool(name="kv", bufs=2) as kvp,
            tc.tile_pool(name="sm", bufs=2) as sm,
            tc.tile_pool(name="ps", bufs=2, space="PSUM") as ps,
            tc.tile_pool(name="ps1", bufs=1, space="PSUM") as ps1,
        ):
            # ---- resident constants ----
            wk2 = const.tile([128, CT, C], BF16, tag="wk2")
            nc.sync.dma_start(
                wk2[:], wk2_d.ap().rearrange("(a p) o -> p a o", p=128))
            wv = const.tile([128, CT, C], BF16, tag="wv")
            nc.sync.dma_start(
                wv[:], wv_d.ap().rearrange("(a p) o -> p a o", p=128))
            wq = const.tile([128, CT, C], BF16, tag="wq")
            nc.sync.dma_start(
                wq[:], wq_d.ap().rearrange("(a p) o -> p a o", p=128))
            wp = const.tile([128, CT, C], BF16, tag="wp")
            nc.sync.dma_start(
                wp[:], wp_d.ap().rearrange("(a p) o -> p a o", p=128))
            xcls = const.tile([128, CT, BPC], BF16, tag="xcls")
            nc.sync.dma_start(
                xcls[:], xcls_d.ap().rearrange("(a p) b -> p a b", p=128))
            qbr = const.tile([BPC, C], F32, tag="qbr")
            nc.sync.dma_start(qbr[:], qb_d.ap())
            pbr = const.tile([BPC, C], F32, tag="pbr")
            nc.sync.dma_start(pbr[:], pb_d.ap())

            identb = const.tile([12, 12], BF16, tag="identb")
            masks.make_identity(nc, identb[:])
            identf = const.tile([BPC, BPC], F32, tag="identf")
            masks.make_identity(nc, identf[:])

            Qblk = const.tile([128, CT, BPC * H], BF16, tag="Qblk")
            nc.vector.memset(Qblk[:], 0.0)
            Wt = const.tile([128, CT, BPC * H], BF16, tag="Wt")
            oT_vec = const.tile([128, CT, BPC], BF16, tag="oT_vec")
            q_sb = const.tile([BPC, C], F32, tag="q_sb")

            # ---- stage B: class-token q for all 8 batches ----
            for ho, hw in C_HALVES:
                pq = ps.tile([BPC, 512], F32, tag="pbh")
                for ci in range(CT):
                    nc.tensor.matmul(
                        pq[:, :hw], xcls[:, ci, :], wq[:, ci, ho:ho + hw],
                        start=(ci == 0), stop=(ci == CT - 1))
                nc.vector.tensor_add(
                    q_sb[:, ho:ho + hw], pq[:, :hw], qbr[:, ho:ho + hw])

            # build block-diagonal Q: Qblk[o, b*12+h] = q_sb[b, o], o in head h
            QblkV = Qblk[:].rearrange("p a (b h) -> p a b h", h=H)
            for ci in range(CT):
                pqt = ps1.tile([128, BPC], F32, tag="ptrf")
                nc.tensor.transpose(
                    pqt[:, :], q_sb[:, ci * 128:(ci + 1) * 128],
                    identf[:BPC, :BPC])
                for j in range(2):
                    h = 2 * ci + j
                    nc.vector.tensor_copy(
                        QblkV[j * 64:(j + 1) * 64, ci, :, h],
                        pqt[j * 64:(j + 1) * 64, :])

            # ---- Wt[c, (b,h)] = wk.T @ Qblk, once for all batches ----
            for cj in range(CT):
                pw = ps1.tile([128, BPC * H], F32, tag="pwt")
                for oj in range(CT):
                    nc.tensor.matmul(
                        pw[:, :], wk2[:, oj, cj * 128:(cj + 1) * 128],
                        Qblk[:, oj, :],
                        start=(oj == 0), stop=(oj == CT - 1))
                nc.vector.tensor_copy(Wt[:, cj, :], pw[:, :])

            # ---- per-batch pipeline ----
            for b in range(BPC):
                xb = xp.tile([128, CT, N], BF16, tag="xb")
                nc.sync.dma_start(
                    xb[:],
                    xT_d.ap()[:, b, :].rearrange("(a p) t -> p a t", p=128))

                # C2: v_b[t, o], t-major (no bias: folded into proj bias)
                vt = kvp.tile([128, NTT, C], BF16, tag="vt")
                for ti, (to, tw) in enumerate(T_TILES):
                    for ho, hw in C_HALVES:
                        pv = ps.tile([128, 512], F32, tag="pkv")
                        for ci in range(CT):
                            nc.tensor.matmul(
                                pv[:tw, :hw],
                                xb[:, ci, to:to + tw],
                                wv[:, ci, ho:ho + hw],
                                start=(ci == 0), stop=(ci == CT - 1))
                        nc.scalar.copy(vt[:tw, ti, ho:ho + hw], pv[:tw, :hw])

                # C3 + C4: scores + softmax (p left unnormalized)
                pss = []
                for no, nw in N_HALVES:
                    s_ps = ps.tile([H, 512], F32, tag="pbh")
                    for ci in range(CT):
                        nc.tensor.matmul(
                            s_ps[:, :nw],
                            Wt[:, ci, b * H:(b + 1) * H],
                            xb[:, ci, no:no + nw],
                            start=(ci == 0), stop=(ci == CT - 1))
                    pss.append(s_ps)

                m1 = sm.tile([H, 1], F32, tag="m1")
                m2 = sm.tile([H, 1], F32, tag="m2")
                nc.vector.reduce_max(m1[:], pss[0][:, :N_HALVES[0][1]], axis=AX.X)
                nc.vector.reduce_max(m2[:], pss[1][:, :N_HALVES[1][1]], axis=AX.X)
                am = sm.tile([H, 1], F32, tag="am")
                nc.vector.tensor_max(am[:], m1[:], m2[:])
                negm = sm.tile([H, 1], F32, tag="negm")
                nc.scalar.mul(negm[:], am[:], -1.0)

                e_bf = sm.tile([H, N], BF16, tag="e_bf")
                sums = []
                for i, (no, nw) in enumerate(N_HALVES):
                    acc = sm.tile([H, 1], F32, tag=f"acc{i}")
                    nc.scalar.activation(
                        e_bf[:, no:no + nw], pss[i][:, :nw], AF.Exp,
                        bias=negm[:], scale=1.0, accum_out=acc[:])
                    sums.append(acc)
                ssum = sm.tile([H, 1], F32, tag="ssum")
                nc.vector.tensor_add(ssum[:], sums[0][:], sums[1][:])
                rden = sm.tile([H, 1], F32, tag="rden")
                nc.vector.reciprocal(rden[:], ssum[:])

                # C5: transpose p -> pT[t, h]
                pT = sm.tile([128, NTT, H], BF16, tag="pT")
                for ti, (to, tw) in enumerate(T_TILES):
                    ppt = ps.tile([128, H], BF16, tag="ptrb")
                    nc.tensor.transpose(
                        ppt[:tw, :], e_bf[:, to:to + tw], identb[:H, :H])
                    nc.vector.tensor_copy(pT[:tw, ti, :], ppt[:tw, :])

                # C6: o_b[h, o] = pT.T @ v  (full 12x768, diag blocks wanted)
                o_tmp = sm.tile([H, C], BF16, tag="o_tmp")
                for ho, hw in C_HALVES:
                    po = ps.tile([H, 512], F32, tag="pbh")
                    for ti, (to, tw) in enumerate(T_TILES):
                        nc.tensor.matmul(
                            po[:, :hw],
                            pT[:tw, ti, :],
                            vt[:tw, ti, ho:ho + hw],
                            start=(ti == 0), stop=(ti == NTT - 1))
                    # normalize by 1/sum while evacuating psum
                    nc.vector.tensor_scalar_mul(
                        o_tmp[:, ho:ho + hw], po[:, :hw], rden[:])

                # C7: extract diagonal blocks, transposed: oT_vec[o, b]
                for ci in range(CT):
                    pot = ps.tile([128, H], BF16, tag="ptrb")
                    nc.tensor.transpose(
                        pot[:, :], o_tmp[:, ci * 128:(ci + 1) * 128],
                        identb[:H, :H])
                    for j in range(2):
                        h = 2 * ci + j
                        nc.vector.tensor_copy(
                            oT_vec[j * 64:(j + 1) * 64, ci, b:b + 1],
                            pot[j * 64:(j + 1) * 64, h:h + 1])

            # ---- C8: proj for all 8 batches ----
            cls_sb = const.tile([BPC, C], F32, tag="cls_sb")
            for ho, hw in C_HALVES:
                pc = ps.tile([BPC, 512], F32, tag="pbh")
                for ci in range(CT):
                    nc.tensor.matmul(
                        pc[:, :hw], oT_vec[:, ci, :], wp[:, ci, ho:ho + hw],
                        start=(ci == 0), stop=(ci == CT - 1))
                nc.vector.tensor_add(
                    cls_sb[:, ho:ho + hw], pc[:, :hw], pbr[:, ho:ho + hw])
            nc.sync.dma_start(cls_d.ap(), cls_sb[:])

    nc.compile()
    return nc


@functools.lru_cache(maxsize=1)
def _module():
    return build_module()


def make_in_maps(x, qkv_w, qkv_b, proj_w, proj_b):
    x = np.asarray(x, dtype=np.float32)
    qkv_w = np.asarray(qkv_w, dtype=np.float32)
    qkv_b = np.asarray(qkv_b, dtype=np.float32)
    proj_w = np.asarray(proj_w, dtype=np.float32)
    proj_b = np.asarray(proj_b, dtype=np.float32)

    wk2 = np.ascontiguousarray(qkv_w[C:2 * C]).astype(NPBF16)       # [o, c]
    wv = np.ascontiguousarray(qkv_w[2 * C:].T).astype(NPBF16)       # [c, o]
    wq = np.ascontiguousarray(qkv_w[:C].T * SCALE).astype(NPBF16)   # [c, o]
    wp = np.ascontiguousarray(proj_w.T).astype(NPBF16)              # [c, o]
    qb = np.tile(qkv_b[:C] * SCALE, (BPC, 1)).astype(np.float32)    # [8, C]
    # v bias contributes exactly (vb @ proj_w.T) to cls; fold into proj bias
    pb_eff = proj_b + qkv_b[2 * C:] @ proj_w.T
    pb = np.tile(pb_eff, (BPC, 1)).astype(np.float32)               # [8, C]

    in_maps = []
    for i in range(NCORES):
        xs = x[i * BPC:(i + 1) * BPC]                               # [8, N, C]
        xT = np.ascontiguousarray(xs.transpose(2, 0, 1)).astype(NPBF16)
        xcls = np.ascontiguousarray(xs[:, 0, :].T).astype(NPBF16)   # [C, 8]
        in_maps.append({
            "xT": xT, "wk2": wk2, "wv": wv, "wq": wq, "wp": wp,
            "xcls": xcls, "qb": qb, "pb": pb,
        })
    return in_maps


def kernel(x, qkv_w, qkv_b, proj_w, proj_b):
    nc = _module()
    in_maps = make_in_maps(x, qkv_w, qkv_b, proj_w, proj_b)
    res = bass_utils.run_bass_kernel_spmd(
        nc, in_maps, core_ids=list(range(NCORES)))
    out = np.array(np.asarray(x), dtype=np.float32, copy=True)
    for i in range(NCORES):
        out[i * BPC:(i + 1) * BPC, 0, :] = res.results[i]["cls"]
    return out


# revision 5
# speedup vs baseline: 1.5715x; 1.0658x over previous
"""ClassAttention kernel for 8x TRN2 NeuronCores.

Reference computation (per batch element):
    qkv = x @ qkv_w.T + qkv_b                      # [N, 3C]
    q, k, v = split(qkv)                           # heads H=12, D=64
    s = softmax((q_cls . k) / sqrt(D))             # class-token query only
    cls = (s @ v) @ proj_w.T + proj_b              # [1, C]
    out = concat([cls, x[1:]])                     # rows 1..N pass through

Only the class token row changes, so the device kernel computes just the
[B, C] cls output; rows 1..N are passed through on the host.

Sharding: data-parallel over batch, 8 batches per core, no collectives.
Compute dtype bf16 (fp32 PSUM accumulation), fp32 output.

Key algebraic restructuring (class-token query only):
  - scores fold the k-projection into a per-batch effective query in
    x-space:  s[b,h,n] = sum_c Wt[c, b*12+h] x[b,n,c]  with
    Wt = wk.T @ blockdiag(q) computed ONCE for all batches (768x96)
    instead of projecting k for every token (eliminates 2/3 of the
    projection FLOPs of a naive k+v implementation).
  - k bias shifts every score of a head by the same constant ->
    cancels in softmax; dropped.
  - v bias contributes sum_n(p)=1 times vb to the attention output ->
    folds into the proj bias on the host: pb_eff = proj_b + vb @ proj_w.T.
  - softmax normalization (1/sum) is applied to the attention OUTPUT
    rows while evacuating the AV psum.

Per-core dataflow (b = 0..8 batches, C=768 in 6 chunks of 128):
  xT[c, b, n]   host-transposed bf16 input
  qc[b, o]    = wqT.T @ xcls + qb    (wq,qb pre-scaled by 1/8 on host)
  Qblk[o, bh] = block-diagonal q     (PE transpose + aligned copies)
  Wt[c, bh]   = wk2.T @ Qblk         (36 matmuls, once)
  v_b[t, o]   = xT_b.T @ wvT         (no bias; psum evacuated on ACT)
  s_b[h, n]   = Wt_b.T @ xT_b
  p_b         = exp(s - max)          (unnormalized, bf16)
  o_b[h, o]   = pT_b.T @ v_b          (12x768; diag blocks are the result)
  oT_vec[o,b] = r * diag-extract      (PE transpose + aligned copies)
  cls[b, :]   = oT.T @ wpT + pb_eff
"""

import functools

import numpy as np
import ml_dtypes

import concourse.bass as bass
import concourse.tile as tile
from concourse import bacc, mybir, masks
from concourse import bass_utils

BF16 = mybir.dt.bfloat16
F32 = mybir.dt.float32
NPBF16 = ml_dtypes.bfloat16

B, N, C = 64, 577, 768
H, D = 12, 64
NCORES = 8
BPC = B // NCORES          # 8 batches per core
CT = C // 128              # 6 chunks of the feature dim
SCALE = D ** -0.5          # folded into wq/qb on the host

# token splits: matmul free dim (<=512 fp32 psum bank), K-chunks (<=128)
N_HALVES = [(0, 289), (289, 288)]
C_HALVES = [(0, 512), (512, 256)]
T_TILES = [(0, 128), (128, 128), (256, 128), (384, 128), (512, 65)]
NTT = len(T_TILES)


def build_module():
    nc = bacc.Bacc("TRN2", target_bir_lowering=False, debug=False)

    xT_d = nc.dram_tensor("xT", [C, BPC, N], BF16, kind="ExternalInput")
    wk2_d = nc.dram_tensor("wk2", [C, C], BF16, kind="ExternalInput")  # [o, c]
    wv_d = nc.dram_tensor("wv", [C, C], BF16, kind="ExternalInput")    # [c, o]
    wq_d = nc.dram_tensor("wq", [C, C], BF16, kind="ExternalInput")    # [c, o]
    wp_d = nc.dram_tensor("wp", [C, C], BF16, kind="ExternalInput")    # [c, o]
    xcls_d = nc.dram_tensor("xcls", [C, BPC], BF16, kind="ExternalInput")
    qb_d = nc.dram_tensor("qb", [BPC, C], F32, kind="ExternalInput")
    pb_d = nc.dram_tensor("pb", [BPC, C], F32, kind="ExternalInput")
    cls_d = nc.dram_tensor("cls", [BPC, C], F32, kind="ExternalOutput")

    AF = mybir.ActivationFunctionType
    AX = mybir.AxisListType

    with tile.TileContext(nc) as tc:
        with (
            tc.tile_pool(name="const", bufs=1) as const,
            tc.tile_pool(name="xp", bufs=3) as xp,
            tc.tile_pool(name="kv", bufs=2) as kvp,
            tc.tile_pool(name="sm", bufs=2) as sm,
            tc.tile_pool(name="ps", bufs=2, space="PSUM") as ps,
            tc.tile_pool(name="ps1", bufs=1, space="PSUM") as ps1,
        ):
            # ---- DMAs, in the order the pipeline consumes them ----
            # (the cost model serializes dma_starts on one channel; wv and
            # the first two x batches come first so PE work starts early)
            wv = const.tile([128, CT, C], BF16, tag="wv")
            nc.sync.dma_start(
                wv[:], wv_d.ap().rearrange("(a p) o -> p a o", p=128))

            xbs = {}

            def load_xb(b):
                xb = xp.tile([128, CT, N], BF16, tag="xb")
                nc.sync.dma_start(
                    xb[:],
                    xT_d.ap()[:, b, :].rearrange("(a p) t -> p a t", p=128))
                xbs[b] = xb

            load_xb(0)
            load_xb(1)

            xcls = const.tile([128, CT, BPC], BF16, tag="xcls")
            nc.sync.dma_start(
                xcls[:], xcls_d.ap().rearrange("(a p) b -> p a b", p=128))
            wq = const.tile([128, CT, C], BF16, tag="wq")
            nc.sync.dma_start(
                wq[:], wq_d.ap().rearrange("(a p) o -> p a o", p=128))
            qbr = const.tile([BPC, C], F32, tag="qbr")
            nc.sync.dma_start(qbr[:], qb_d.ap())
            wk2 = const.tile([128, CT, C], BF16, tag="wk2")
            nc.sync.dma_start(
                wk2[:], wk2_d.ap().rearrange("(a p) o -> p a o", p=128))
            pbr = const.tile([BPC, C], F32, tag="pbr")
            nc.sync.dma_start(pbr[:], pb_d.ap())
            wp = const.tile([128, CT, C], BF16, tag="wp")
            nc.sync.dma_start(
                wp[:], wp_d.ap().rearrange("(a p) o -> p a o", p=128))

            identb = const.tile([12, 12], BF16, tag="identb")
            masks.make_identity(nc, identb[:])
            identf = const.tile([BPC, BPC], F32, tag="identf")
            masks.make_identity(nc, identf[:])

            Qblk = const.tile([128, CT, BPC * H], BF16, tag="Qblk")
            nc.vector.memset(Qblk[:], 0.0)
            Wt = const.tile([128, CT, BPC * H], BF16, tag="Wt")
            oT_vec = const.tile([128, CT, BPC], BF16, tag="oT_vec")
            q_sb = const.tile([BPC, C], F32, tag="q_sb")

            def emit_q_wt():
                # class-token q for all 8 batches
                for ho, hw in C_HALVES:
                    pq = ps.tile([BPC, 512], F32, tag="pbh")
                    for ci in range(CT):
                        nc.tensor.matmul(
                            pq[:, :hw], xcls[:, ci, :], wq[:, ci, ho:ho + hw],
                            start=(ci == 0), stop=(ci == CT - 1))
                    nc.vector.tensor_add(
                        q_sb[:, ho:ho + hw], pq[:, :hw], qbr[:, ho:ho + hw])

                # block-diagonal Q: Qblk[o, b*12+h] = q_sb[b, o], o in head h
                QblkV = Qblk[:].rearrange("p a (b h) -> p a b h", h=H)
                for ci in range(CT):
                    pqt = ps1.tile([128, BPC], F32, tag="ptrf")
                    nc.tensor.transpose(
                        pqt[:, :], q_sb[:, ci * 128:(ci + 1) * 128],
                        identf[:BPC, :BPC])
                    for j in range(2):
                        h = 2 * ci + j
                        nc.vector.tensor_copy(
                            QblkV[j * 64:(j + 1) * 64, ci, :, h],
                            pqt[j * 64:(j + 1) * 64, :])

                # Wt[c, (b,h)] = wk.T @ Qblk, once for all batches
                for cj in range(CT):
                    pw = ps1.tile([128, BPC * H], F32, tag="pwt")
                    for oj in range(CT):
                        nc.tensor.matmul(
                            pw[:, :], wk2[:, oj, cj * 128:(cj + 1) * 128],
                            Qblk[:, oj, :],
                            start=(oj == 0), stop=(oj == CT - 1))
                    nc.vector.tensor_copy(Wt[:, cj, :], pw[:, :])

            # ---- per-batch pipeline ----
            for b in range(BPC):
                if b + 2 < BPC:
                    load_xb(b + 2)
                xb = xbs.pop(b)

                def emit_scores():
                    pss = []
                    for no, nw in N_HALVES:
                        s_ps = ps.tile([H, 512], F32, tag="pbh")
                        for ci in range(CT):
                            nc.tensor.matmul(
                                s_ps[:, :nw],
                                Wt[:, ci, b * H:(b + 1) * H],
                                xb[:, ci, no:no + nw],
                                start=(ci == 0), stop=(ci == CT - 1))
                        pss.append(s_ps)
                    return pss

                def emit_v():
                    # v_b[t, o], t-major (no bias: folded into proj bias);
                    # psum evacuation split between ACT (512) and DVE (256)
                    vt = kvp.tile([128, NTT, C], BF16, tag="vt")
                    for ti, (to, tw) in enumerate(T_TILES):
                        for ho, hw in C_HALVES:
                            pv = ps.tile([128, 512], F32, tag="pkv")
                            for ci in range(CT):
                                nc.tensor.matmul(
                                    pv[:tw, :hw],
                                    xb[:, ci, to:to + tw],
                                    wv[:, ci, ho:ho + hw],
                                    start=(ci == 0), stop=(ci == CT - 1))
                            if hw == 512:
                                nc.scalar.copy(
                                    vt[:tw, ti, ho:ho + hw], pv[:tw, :hw])
                            else:
                                nc.vector.tensor_copy(
                                    vt[:tw, ti, ho:ho + hw], pv[:tw, :hw])
                    return vt

                # batch 0: Wt isn't ready until the wk2 DMA lands, so run v
                # first; later batches run scores first so the softmax chain
                # overlaps the v matmuls.
                if b == 0:
                    vt = emit_v()
                    emit_q_wt()
                    pss = emit_scores()
                else:
                    pss = emit_scores()
                    vt = emit_v()

                m1 = sm.tile([H, 1], F32, tag="m1")
                m2 = sm.tile([H, 1], F32, tag="m2")
                nc.vector.reduce_max(m1[:], pss[0][:, :N_HALVES[0][1]], axis=AX.X)
                nc.vector.reduce_max(m2[:], pss[1][:, :N_HALVES[1][1]], axis=AX.X)
                am = sm.tile([H, 1], F32, tag="am")
                nc.vector.tensor_max(am[:], m1[:], m2[:])
                negm = sm.tile([H, 1], F32, tag="negm")
                nc.scalar.mul(negm[:], am[:], -1.0)

                e_bf = sm.tile([H, N], BF16, tag="e_bf")
                sums = []
                for i, (no, nw) in enumerate(N_HALVES):
                    acc = sm.tile([H, 1], F32, tag=f"acc{i}")
                    nc.scalar.activation(
                        e_bf[:, no:no + nw], pss[i][:, :nw], AF.Exp,
                        bias=negm[:], scale=1.0, accum_out=acc[:])
                    sums.append(acc)
                ssum = sm.tile([H, 1], F32, tag="ssum")
                nc.vector.tensor_add(ssum[:], sums[0][:], sums[1][:])
                rden = sm.tile([H, 1], F32, tag="rden")
                nc.vector.reciprocal(rden[:], ssum[:])

                # C5: transpose p -> pT[t, h]
                pT = sm.tile([128, NTT, H], BF16, tag="pT")
                for ti, (to, tw) in enumerate(T_TILES):
                    ppt = ps.tile([128, H], BF16, tag="ptrb")
                    nc.tensor.transpose(
                        ppt[:tw, :], e_bf[:, to:to + tw], identb[:H, :H])
                    nc.vector.tensor_copy(pT[:tw, ti, :], ppt[:tw, :])

                # C6: o_b[h, o] = pT.T @ v  (full 12x768, diag blocks wanted)
                o_tmp = sm.tile([H, C], BF16, tag="o_tmp")
                for ho, hw in C_HALVES:
                    po = ps.tile([H, 512], F32, tag="pbh")
                    for ti, (to, tw) in enumerate(T_TILES):
                        nc.tensor.matmul(
                            po[:, :hw],
                            pT[:tw, ti, :],
                            vt[:tw, ti, ho:ho + hw],
                            start=(ti == 0), stop=(ti == NTT - 1))
                    # normalize by 1/sum while evacuating psum
                    nc.vector.tensor_scalar_mul(
                        o_tmp[:, ho:ho + hw], po[:, :hw], rden[:])

                # C7: extract diagonal blocks, transposed: oT_vec[o, b]
                for ci in range(CT):
                    pot = ps.tile([128, H], BF16, tag="ptrb")
                    nc.tensor.transpose(
                        pot[:, :], o_tmp[:, ci * 128:(ci + 1) * 128],
                        identb[:H, :H])
                    for j in range(2):
                        h = 2 * ci + j
                        nc.vector.tensor_copy(
                            oT_vec[j * 64:(j + 1) * 64, ci, b:b + 1],
                            pot[j * 64:(j + 1) * 64, h:h + 1])

            # ---- C8: proj for all 8 batches ----
            cls_sb = const.tile([BPC, C], F32, tag="cls_sb")
            for ho, hw in C_HALVES:
                pc = ps.tile([BPC, 512], F32, tag="pbh")
                for ci in range(CT):
                    nc.tensor.matmul(
                        pc[:, :hw], oT_vec[:, ci, :], wp[:, ci, ho:ho + hw],
                        start=(ci == 0), stop=(ci == CT - 1))
                nc.vector.tensor_add(
                    cls_sb[:, ho:ho + hw], pc[:, :hw], pbr[:, ho:ho + hw])
            nc.sync.dma_start(cls_d.ap(), cls_sb[:])

    nc.compile()
    return nc


@functools.lru_cache(maxsize=1)
def _module():
    return build_module()


def make_in_maps(x, qkv_w, qkv_b, proj_w, proj_b):
    x = np.asarray(x, dtype=np.float32)
    qkv_w = np.asarray(qkv_w, dtype=np.float32)
    qkv_b = np.asarray(qkv_b, dtype=np.float32)
    proj_w = np.asarray(proj_w, dtype=np.float32)
    proj_b = np.asarray(proj_b, dtype=np.float32)

    wk2 = np.ascontiguousarray(qkv_w[C:2 * C]).astype(NPBF16)       # [o, c]
    wv = np.ascontiguousarray(qkv_w[2 * C:].T).astype(NPBF16)       # [c, o]
    wq = np.ascontiguousarray(qkv_w[:C].T * SCALE).astype(NPBF16)   # [c, o]
    wp = np.ascontiguousarray(proj_w.T).astype(NPBF16)              # [c, o]
    qb = np.tile(qkv_b[:C] * SCALE, (BPC, 1)).astype(np.float32)    # [8, C]
    # v bias contributes exactly (vb @ proj_w.T) to cls; fold into proj bias
    pb_eff = proj_b + qkv_b[2 * C:] @ proj_w.T
    pb = np.tile(pb_eff, (BPC, 1)).astype(np.float32)               # [8, C]

    in_maps = []
    for i in range(NCORES):
        xs = x[i * BPC:(i + 1) * BPC]                               # [8, N, C]
        xT = np.ascontiguousarray(xs.transpose(2, 0, 1)).astype(NPBF16)
        xcls = np.ascontiguousarray(xs[:, 0, :].T).astype(NPBF16)   # [C, 8]
        in_maps.append({
            "xT": xT, "wk2": wk2, "wv": wv, "wq": wq, "wp": wp,
            "xcls": xcls, "qb": qb, "pb": pb,
        })
    return in_maps


def kernel(x, qkv_w, qkv_b, proj_w, proj_b):
    nc = _module()
    in_maps = make_in_maps(x, qkv_w, qkv_b, proj_w, proj_b)
    res = bass_utils.run_bass_kernel_spmd(
        nc, in_maps, core_ids=list(range(NCORES)))
    out = np.array(np.asarray(x), dtype=np.float32, copy=True)
    for i in range(NCORES):
        out[i * BPC:(i + 1) * BPC, 0, :] = res.results[i]["cls"]
    return out


# revision 6
# speedup vs baseline: 1.5892x; 1.0113x over previous
"""ClassAttention kernel for 8x TRN2 NeuronCores.

Reference computation (per batch element):
    qkv = x @ qkv_w.T + qkv_b                      # [N, 3C]
    q, k, v = split(qkv)                           # heads H=12, D=64
    s = softmax((q_cls . k) / sqrt(D))             # class-token query only
    cls = (s @ v) @ proj_w.T + proj_b              # [1, C]
    out = concat([cls, x[1:]])                     # rows 1..N pass through

Only the class token row changes, so the device kernel computes just the
[B, C] cls output; rows 1..N are passed through on the host.

Sharding: data-parallel over batch, 8 batches per core, no collectives.
Compute dtype bf16 (fp32 PSUM accumulation), fp32 output.

Key algebraic restructuring (class-token query only):
  - scores fold the k-projection into a per-batch effective query in
    x-space:  s[b,h,n] = sum_c Wt[c, b*12+h] x[b,n,c]  with
    Wt = wk.T @ blockdiag(q) computed ONCE for all batches (768x96)
    instead of projecting k for every token (eliminates 2/3 of the
    projection FLOPs of a naive k+v implementation).
  - k bias shifts every score of a head by the same constant ->
    cancels in softmax; dropped.
  - v bias contributes sum_n(p)=1 times vb to the attention output ->
    folds into the proj bias on the host: pb_eff = proj_b + vb @ proj_w.T.
  - softmax normalization (1/sum) is applied to the attention OUTPUT
    rows while evacuating the AV psum.

Per-core dataflow (b = 0..8 batches, C=768 in 6 chunks of 128):
  xT[c, b, n]   host-transposed bf16 input
  qc[b, o]    = wqT.T @ xcls + qb    (wq,qb pre-scaled by 1/8 on host)
  Qblk[o, bh] = block-diagonal q     (PE transpose + aligned copies)
  Wt[c, bh]   = wk2.T @ Qblk         (36 matmuls, once)
  v_b[t, o]   = xT_b.T @ wvT         (no bias; psum evacuated on ACT)
  s_b[h, n]   = Wt_b.T @ xT_b
  p_b         = exp(s - max)          (unnormalized, bf16)
  o_b[h, o]   = pT_b.T @ v_b          (12x768; diag blocks are the result)
  oT_vec[o,b] = r * diag-extract      (PE transpose + aligned copies)
  cls[b, :]   = oT.T @ wpT + pb_eff
"""

import functools

import numpy as np
import ml_dtypes

import concourse.bass as bass
import concourse.tile as tile
from concourse import bacc, mybir, masks
from concourse import bass_utils

BF16 = mybir.dt.bfloat16
F32 = mybir.dt.float32
NPBF16 = ml_dtypes.bfloat16

B, N, C = 64, 577, 768
H, D = 12, 64
NCORES = 8
BPC = B // NCORES          # 8 batches per core
CT = C // 128              # 6 chunks of the feature dim
SCALE = D ** -0.5          # folded into wq/qb on the host

# token splits: matmul free dim (<=512 fp32 psum bank), K-chunks (<=128)
N_HALVES = [(0, 289), (289, 288)]
C_HALVES = [(0, 512), (512, 256)]
T_TILES = [(0, 128), (128, 128), (256, 128), (384, 128), (512, 65)]
NTT = len(T_TILES)


def build_module():
    nc = bacc.Bacc("TRN2", target_bir_lowering=False, debug=False)

    xT_d = nc.dram_tensor("xT", [C, BPC, N], BF16, kind="ExternalInput")
    wk2_d = nc.dram_tensor("wk2", [C, C], BF16, kind="ExternalInput")  # [o, c]
    wv_d = nc.dram_tensor("wv", [C, C], BF16, kind="ExternalInput")    # [c, o]
    wq_d = nc.dram_tensor("wq", [C, C], BF16, kind="ExternalInput")    # [c, o]
    wp_d = nc.dram_tensor("wp", [C, C], BF16, kind="ExternalInput")    # [c, o]
    xcls_d = nc.dram_tensor("xcls", [C, BPC], BF16, kind="ExternalInput")
    qb_d = nc.dram_tensor("qb", [BPC, C], F32, kind="ExternalInput")
    pb_d = nc.dram_tensor("pb", [BPC, C], F32, kind="ExternalInput")
    cls_d = nc.dram_tensor("cls", [BPC, C], F32, kind="ExternalOutput")

    AF = mybir.ActivationFunctionType
    AX = mybir.AxisListType

    with tile.TileContext(nc) as tc:
        with (
            tc.tile_pool(name="const", bufs=1) as const,
            tc.tile_pool(name="xp", bufs=3) as xp,
            tc.tile_pool(name="kv", bufs=2) as kvp,
            tc.tile_pool(name="sm", bufs=2) as sm,
            tc.tile_pool(name="ps", bufs=2, space="PSUM") as ps,
            tc.tile_pool(name="ps1", bufs=1, space="PSUM") as ps1,
        ):
            # ---- DMAs, in the order the pipeline consumes them ----
            # (the cost model serializes dma_starts on one channel; wv and
            # the first two x batches come first so PE work starts early)
            wv = const.tile([128, CT, C], BF16, tag="wv")
            nc.sync.dma_start(
                wv[:], wv_d.ap().rearrange("(a p) o -> p a o", p=128))

            xbs = {}

            def load_xb(b):
                xb = xp.tile([128, CT, N], BF16, tag="xb")
                nc.sync.dma_start(
                    xb[:],
                    xT_d.ap()[:, b, :].rearrange("(a p) t -> p a t", p=128))
                xbs[b] = xb

            load_xb(0)
            load_xb(1)

            xcls = const.tile([128, CT, BPC], BF16, tag="xcls")
            nc.sync.dma_start(
                xcls[:], xcls_d.ap().rearrange("(a p) b -> p a b", p=128))
            wq = const.tile([128, CT, C], BF16, tag="wq")
            nc.sync.dma_start(
                wq[:], wq_d.ap().rearrange("(a p) o -> p a o", p=128))
            qbr = const.tile([BPC, C], F32, tag="qbr")
            nc.sync.dma_start(qbr[:], qb_d.ap())
            wk2 = const.tile([128, CT, C], BF16, tag="wk2")
            nc.sync.dma_start(
                wk2[:], wk2_d.ap().rearrange("(a p) o -> p a o", p=128))
            pbr = const.tile([BPC, C], F32, tag="pbr")
            nc.sync.dma_start(pbr[:], pb_d.ap())
            wp = const.tile([128, CT, C], BF16, tag="wp")
            nc.sync.dma_start(
                wp[:], wp_d.ap().rearrange("(a p) o -> p a o", p=128))

            identb = const.tile([12, 12], BF16, tag="identb")
            masks.make_identity(nc, identb[:])
            identf = const.tile([BPC, BPC], F32, tag="identf")
            masks.make_identity(nc, identf[:])

            Qblk = const.tile([128, CT, BPC * H], BF16, tag="Qblk")
            nc.vector.memset(Qblk[:], 0.0)
            Wt = const.tile([128, CT, BPC * H], BF16, tag="Wt")
            oT_vec = const.tile([128, CT, BPC], BF16, tag="oT_vec")
            q_sb = const.tile([BPC, C], F32, tag="q_sb")

            def emit_q_wt():
                # class-token q for all 8 batches
                for ho, hw in C_HALVES:
                    pq = ps.tile([BPC, 512], F32, tag="pav")
                    for ci in range(CT):
                        nc.tensor.matmul(
                            pq[:, :hw], xcls[:, ci, :], wq[:, ci, ho:ho + hw],
                            start=(ci == 0), stop=(ci == CT - 1))
                    nc.vector.tensor_add(
                        q_sb[:, ho:ho + hw], pq[:, :hw], qbr[:, ho:ho + hw])

                # block-diagonal Q: Qblk[o, b*12+h] = q_sb[b, o], o in head h
                QblkV = Qblk[:].rearrange("p a (b h) -> p a b h", h=H)
                for ci in range(CT):
                    pqt = ps1.tile([128, BPC], F32, tag="pwt")
                    nc.tensor.transpose(
                        pqt[:, :], q_sb[:, ci * 128:(ci + 1) * 128],
                        identf[:BPC, :BPC])
                    for j in range(2):
                        h = 2 * ci + j
                        nc.vector.tensor_copy(
                            QblkV[j * 64:(j + 1) * 64, ci, :, h],
                            pqt[j * 64:(j + 1) * 64, :])

                # Wt[c, (b,h)] = wk.T @ Qblk, once for all batches
                for cj in range(CT):
                    pw = ps1.tile([128, BPC * H], F32, tag="pwt")
                    for oj in range(CT):
                        nc.tensor.matmul(
                            pw[:, :], wk2[:, oj, cj * 128:(cj + 1) * 128],
                            Qblk[:, oj, :],
                            start=(oj == 0), stop=(oj == CT - 1))
                    nc.vector.tensor_copy(Wt[:, cj, :], pw[:, :])

            # ---- per-batch pipeline ----
            for b in range(BPC):
                if b + 2 < BPC:
                    load_xb(b + 2)
                xb = xbs.pop(b)

                def emit_scores():
                    pss = []
                    for no, nw in N_HALVES:
                        s_ps = ps.tile([H, 512], F32, tag="psc")
                        for ci in range(CT):
                            nc.tensor.matmul(
                                s_ps[:, :nw],
                                Wt[:, ci, b * H:(b + 1) * H],
                                xb[:, ci, no:no + nw],
                                start=(ci == 0), stop=(ci == CT - 1))
                        pss.append(s_ps)
                    return pss

                def emit_v():
                    # v_b[t, o], t-major (no bias: folded into proj bias);
                    # psum evacuation split between ACT (512) and DVE (256)
                    vt = kvp.tile([128, NTT, C], BF16, tag="vt")
                    for ti, (to, tw) in enumerate(T_TILES):
                        for ho, hw in C_HALVES:
                            pv = ps.tile([128, 512], F32, tag="pkv")
                            for ci in range(CT):
                                nc.tensor.matmul(
                                    pv[:tw, :hw],
                                    xb[:, ci, to:to + tw],
                                    wv[:, ci, ho:ho + hw],
                                    start=(ci == 0), stop=(ci == CT - 1))
                            if hw == 512:
                                nc.scalar.copy(
                                    vt[:tw, ti, ho:ho + hw], pv[:tw, :hw])
                            else:
                                nc.vector.tensor_copy(
                                    vt[:tw, ti, ho:ho + hw], pv[:tw, :hw])
                    return vt

                # batch 0: Wt isn't ready until the wk2 DMA lands, so run v
                # first; later batches run scores first so the softmax chain
                # overlaps the v matmuls.
                if b == 0:
                    vt = emit_v()
                    emit_q_wt()
                    pss = emit_scores()
                else:
                    pss = emit_scores()
                    vt = emit_v()

                m1 = sm.tile([H, 1], F32, tag="m1")
                m2 = sm.tile([H, 1], F32, tag="m2")
                nc.vector.reduce_max(m1[:], pss[0][:, :N_HALVES[0][1]], axis=AX.X)
                nc.vector.reduce_max(m2[:], pss[1][:, :N_HALVES[1][1]], axis=AX.X)
                am = sm.tile([H, 1], F32, tag="am")
                nc.vector.tensor_max(am[:], m1[:], m2[:])
                negm = sm.tile([H, 1], F32, tag="negm")
                nc.scalar.mul(negm[:], am[:], -1.0)

                e_bf = sm.tile([H, N], BF16, tag="e_bf")
                sums = []
                for i, (no, nw) in enumerate(N_HALVES):
                    acc = sm.tile([H, 1], F32, tag=f"acc{i}")
                    nc.scalar.activation(
                        e_bf[:, no:no + nw], pss[i][:, :nw], AF.Exp,
                        bias=negm[:], scale=1.0, accum_out=acc[:])
                    sums.append(acc)
                ssum = sm.tile([H, 1], F32, tag="ssum")
                nc.vector.tensor_add(ssum[:], sums[0][:], sums[1][:])
                rden = sm.tile([H, 1], F32, tag="rden")
                nc.vector.reciprocal(rden[:], ssum[:])

                # C5: transpose p -> pT[t, h]
                pT = sm.tile([128, NTT, H], BF16, tag="pT")
                for ti, (to, tw) in enumerate(T_TILES):
                    ppt = ps1.tile([128, H], BF16, tag="ptrb")
                    nc.tensor.transpose(
                        ppt[:tw, :], e_bf[:, to:to + tw], identb[:H, :H])
                    nc.vector.tensor_copy(pT[:tw, ti, :], ppt[:tw, :])

                # C6: o_b[h, o] = pT.T @ v  (full 12x768, diag blocks wanted)
                o_tmp = sm.tile([H, C], BF16, tag="o_tmp")
                for ho, hw in C_HALVES:
                    po = ps.tile([H, 512], F32, tag="pav")
                    for ti, (to, tw) in enumerate(T_TILES):
                        nc.tensor.matmul(
                            po[:, :hw],
                            pT[:tw, ti, :],
                            vt[:tw, ti, ho:ho + hw],
                            start=(ti == 0), stop=(ti == NTT - 1))
                    # normalize by 1/sum while evacuating psum
                    nc.vector.tensor_scalar_mul(
                        o_tmp[:, ho:ho + hw], po[:, :hw], rden[:])

                # C7: extract diagonal blocks, transposed: oT_vec[o, b]
                for ci in range(CT):
                    pot = ps1.tile([128, H], BF16, tag="ptrb")
                    nc.tensor.transpose(
                        pot[:, :], o_tmp[:, ci * 128:(ci + 1) * 128],
                        identb[:H, :H])
                    for j in range(2):
                        h = 2 * ci + j
                        nc.vector.tensor_copy(
                            oT_vec[j * 64:(j + 1) * 64, ci, b:b + 1],
                            pot[j * 64:(j + 1) * 64, h:h + 1])

            # ---- C8: proj for all 8 batches ----
            cls_sb = const.tile([BPC, C], F32, tag="cls_sb")
            for ho, hw in C_HALVES:
                pc = ps.tile([BPC, 512], F32, tag="pav")
                for ci in range(CT):
                    nc.tensor.matmul(
                        pc[:, :hw], oT_vec[:, ci, :], wp[:, ci, ho:ho + hw],
                        start=(ci == 0), stop=(ci == CT - 1))
                nc.vector.tensor_add(
                    cls_sb[:, ho:ho + hw], pc[:, :hw], pbr[:, ho:ho + hw])
            nc.sync.dma_start(cls_d.ap(), cls_sb[:])

    nc.compile()
    return nc


@functools.lru_cache(maxsize=1)
def _module():
    return build_module()


def make_in_maps(x, qkv_w, qkv_b, proj_w, proj_b):
    x = np.asarray(x, dtype=np.float32)
    qkv_w = np.asarray(qkv_w, dtype=np.float32)
    qkv_b = np.asarray(qkv_b, dtype=np.float32)
    proj_w = np.asarray(proj_w, dtype=np.float32)
    proj_b = np.asarray(proj_b, dtype=np.float32)

    wk2 = np.ascontiguousarray(qkv_w[C:2 * C]).astype(NPBF16)       # [o, c]
    wv = np.ascontiguousarray(qkv_w[2 * C:].T).astype(NPBF16)       # [c, o]
    wq = np.ascontiguousarray(qkv_w[:C].T * SCALE).astype(NPBF16)   # [c, o]
    wp = np.ascontiguousarray(proj_w.T).astype(NPBF16)              # [c, o]
    qb = np.tile(qkv_b[:C] * SCALE, (BPC, 1)).astype(np.float32)    # [8, C]
    # v bias contributes exactly (vb @ proj_w.T) to cls; fold into proj bias
    pb_eff = proj_b + qkv_b[2 * C:] @ proj_w.T
    pb = np.tile(pb_eff, (BPC, 1)).astype(np.float32)               # [8, C]

    in_maps = []
    for i in range(NCORES):
        xs = x[i * BPC:(i + 1) * BPC]                               # [8, N, C]
        xT = np.ascontiguousarray(xs.transpose(2, 0, 1)).astype(NPBF16)
        xcls = np.ascontiguousarray(xs[:, 0, :].T).astype(NPBF16)   # [C, 8]
        in_maps.append({
            "xT": xT, "wk2": wk2, "wv": wv, "wq": wq, "wp": wp,
            "xcls": xcls, "qb": qb, "pb": pb,
        })
    return in_maps


def kernel(x, qkv_w, qkv_b, proj_w, proj_b):
    nc = _module()
    in_maps = make_in_maps(x, qkv_w, qkv_b, proj_w, proj_b)
    res = bass_utils.run_bass_kernel_spmd(
        nc, in_maps, core_ids=list(range(NCORES)))
    out = np.array(np.asarray(x), dtype=np.float32, copy=True)
    for i in range(NCORES):
        out[i * BPC:(i + 1) * BPC, 0, :] = res.results[i]["cls"]
    return out


# revision 8
# speedup vs baseline: 1.6483x; 1.0372x over previous
"""ClassAttention kernel for 8x TRN2 NeuronCores.

Reference computation (per batch element):
    qkv = x @ qkv_w.T + qkv_b                      # [N, 3C]
    q, k, v = split(qkv)                           # heads H=12, D=64
    s = softmax((q_cls . k) / sqrt(D))             # class-token query only
    cls = (s @ v) @ proj_w.T + proj_b              # [1, C]
    out = concat([cls, x[1:]])                     # rows 1..N pass through

Only the class token row changes, so the device kernel computes just the
[B, C] cls output; rows 1..N are passed through on the host.

Sharding: data-parallel over batch, 8 batches per core, no collectives.
Compute dtype bf16 (fp32 PSUM accumulation), fp32 output.

Key algebraic restructuring (class-token query only):
  - scores fold the k-projection into a per-batch effective query in
    x-space:  s[b,h,n] = sum_c Wt[c, b*12+h] x[b,n,c]  with
    Wt = wk.T @ blockdiag(q) computed ONCE for all batches (768x96)
    instead of projecting k for every token (eliminates 2/3 of the
    projection FLOPs of a naive k+v implementation).
  - k bias shifts every score of a head by the same constant ->
    cancels in softmax; dropped.
  - v bias contributes sum_n(p)=1 times vb to the attention output ->
    folds into the proj bias on the host: pb_eff = proj_b + vb @ proj_w.T.
  - softmax normalization (1/sum) is applied to the attention OUTPUT
    rows while evacuating the AV psum.

Per-core dataflow (b = 0..8 batches, C=768 in 6 chunks of 128):
  xT[c, b, n]   host-transposed bf16 input
  qc[b, o]    = wqT.T @ xcls + qb    (wq,qb pre-scaled by 1/8 on host)
  Qblk[o, bh] = block-diagonal q     (PE transpose + aligned copies)
  Wt[c, bh]   = wk2.T @ Qblk         (36 matmuls, once)
  v_b[t, o]   = xT_b.T @ wvT         (no bias; psum evacuated on ACT)
  s_b[h, n]   = Wt_b.T @ xT_b
  p_b         = exp(s - max)          (unnormalized, bf16)
  o_b[h, o]   = pT_b.T @ v_b          (12x768; diag blocks are the result)
  oT_vec[o,b] = r * diag-extract      (PE transpose + aligned copies)
  cls[b, :]   = oT.T @ wpT + pb_eff
"""

import functools

import numpy as np
import ml_dtypes

import concourse.bass as bass
import concourse.tile as tile
from concourse import bacc, mybir, masks
from concourse import bass_utils

BF16 = mybir.dt.bfloat16
F32 = mybir.dt.float32
NPBF16 = ml_dtypes.bfloat16

B, N, C = 64, 577, 768
H, D = 12, 64
NCORES = 8
BPC = B // NCORES          # 8 batches per core
CT = C // 128              # 6 chunks of the feature dim
SCALE = D ** -0.5          # folded into wq/qb on the host

# token splits: matmul free dim (<=512 fp32 psum bank), K-chunks (<=128)
N_HALVES = [(0, 289), (289, 288)]
C_HALVES = [(0, 512), (512, 256)]
T_TILES = [(0, 128), (128, 128), (256, 128), (384, 128), (512, 65)]
NTT = len(T_TILES)


def build_module():
    nc = bacc.Bacc("TRN2", target_bir_lowering=False, debug=False)

    xT_d = nc.dram_tensor("xT", [C, BPC, N], BF16, kind="ExternalInput")
    wk2_d = nc.dram_tensor("wk2", [C, C], BF16, kind="ExternalInput")  # [o, c]
    wv_d = nc.dram_tensor("wv", [C, C], BF16, kind="ExternalInput")    # [c, o]
    wq_d = nc.dram_tensor("wq", [C, C], BF16, kind="ExternalInput")    # [c, o]
    wp_d = nc.dram_tensor("wp", [C, C], BF16, kind="ExternalInput")    # [c, o]
    xcls_d = nc.dram_tensor("xcls", [C, BPC], BF16, kind="ExternalInput")
    qb_d = nc.dram_tensor("qb", [BPC, C], F32, kind="ExternalInput")
    pb_d = nc.dram_tensor("pb", [BPC, C], F32, kind="ExternalInput")
    cls_d = nc.dram_tensor("cls", [BPC, C], F32, kind="ExternalOutput")

    AF = mybir.ActivationFunctionType
    AX = mybir.AxisListType

    with tile.TileContext(nc) as tc:
        with (
            tc.tile_pool(name="const", bufs=1) as const,
            tc.tile_pool(name="xp", bufs=3) as xp,
            tc.tile_pool(name="kv", bufs=2) as kvp,
            tc.tile_pool(name="sm", bufs=2) as sm,
            tc.tile_pool(name="ps", bufs=2, space="PSUM") as ps,
            tc.tile_pool(name="ps1", bufs=1, space="PSUM") as ps1,
        ):
            # ---- DMAs, in the order the pipeline consumes them ----
            # (the cost model serializes dma_starts on one channel; xb0 and
            # the first wv half come first so PE work starts early)
            xbs = {}

            def load_xb(b):
                xb = xp.tile([128, CT, N], BF16, tag="xb")
                nc.sync.dma_start(
                    xb[:],
                    xT_d.ap()[:, b, :].rearrange("(a p) t -> p a t", p=128))
                xbs[b] = xb

            load_xb(0)
            wv = const.tile([128, CT, C], BF16, tag="wv")
            wvr = wv_d.ap().rearrange("(a p) o -> p a o", p=128)
            for ho, hw in C_HALVES:
                nc.sync.dma_start(wv[:, :, ho:ho + hw], wvr[:, :, ho:ho + hw])
            load_xb(1)

            xcls = const.tile([128, CT, BPC], BF16, tag="xcls")
            nc.sync.dma_start(
                xcls[:], xcls_d.ap().rearrange("(a p) b -> p a b", p=128))
            wq = const.tile([128, CT, C], BF16, tag="wq")
            nc.sync.dma_start(
                wq[:], wq_d.ap().rearrange("(a p) o -> p a o", p=128))
            qbr = const.tile([BPC, C], F32, tag="qbr")
            nc.sync.dma_start(qbr[:], qb_d.ap())
            wk2 = const.tile([128, CT, C], BF16, tag="wk2")
            nc.sync.dma_start(
                wk2[:], wk2_d.ap().rearrange("(a p) o -> p a o", p=128))
            pbr = const.tile([BPC, C], F32, tag="pbr")
            nc.sync.dma_start(pbr[:], pb_d.ap())
            wp = const.tile([128, CT, C], BF16, tag="wp")
            nc.sync.dma_start(
                wp[:], wp_d.ap().rearrange("(a p) o -> p a o", p=128))

            identb = const.tile([12, 12], BF16, tag="identb")
            masks.make_identity(nc, identb[:])
            identf = const.tile([BPC, BPC], F32, tag="identf")
            masks.make_identity(nc, identf[:])

            Qblk = const.tile([128, CT, BPC * H], BF16, tag="Qblk")
            nc.vector.memset(Qblk[:], 0.0)
            Wt = const.tile([128, CT, BPC * H], BF16, tag="Wt")
            oT_vec = const.tile([128, CT, BPC], BF16, tag="oT_vec")
            q_sb = const.tile([BPC, C], F32, tag="q_sb")

            def emit_q_wt():
                # class-token q for all 8 batches
                for ho, hw in C_HALVES:
                    pq = ps.tile([BPC, 512], F32, tag="pav")
                    for ci in range(CT):
                        nc.tensor.matmul(
                            pq[:, :hw], xcls[:, ci, :], wq[:, ci, ho:ho + hw],
                            start=(ci == 0), stop=(ci == CT - 1))
                    nc.vector.tensor_add(
                        q_sb[:, ho:ho + hw], pq[:, :hw], qbr[:, ho:ho + hw])

                # block-diagonal Q: Qblk[o, b*12+h] = q_sb[b, o], o in head h
                QblkV = Qblk[:].rearrange("p a (b h) -> p a b h", h=H)
                for ci in range(CT):
                    pqt = ps1.tile([128, BPC], F32, tag="pwt")
                    nc.tensor.transpose(
                        pqt[:, :], q_sb[:, ci * 128:(ci + 1) * 128],
                        identf[:BPC, :BPC])
                    for j in range(2):
                        h = 2 * ci + j
                        nc.vector.tensor_copy(
                            QblkV[j * 64:(j + 1) * 64, ci, :, h],
                            pqt[j * 64:(j + 1) * 64, :])

                # Wt[c, (b,h)] = wk.T @ Qblk, once for all batches
                for cj in range(CT):
                    pw = ps1.tile([128, BPC * H], F32, tag="pwt")
                    for oj in range(CT):
                        nc.tensor.matmul(
                            pw[:, :], wk2[:, oj, cj * 128:(cj + 1) * 128],
                            Qblk[:, oj, :],
                            start=(oj == 0), stop=(oj == CT - 1))
                    nc.vector.tensor_copy(Wt[:, cj, :], pw[:, :])

            # ---- per-batch pipeline ----
            for b in range(BPC):
                if b + 2 < BPC:
                    load_xb(b + 2)
                xb = xbs.pop(b)

                def emit_scores():
                    pss = []
                    for no, nw in N_HALVES:
                        s_ps = ps.tile([H, 512], F32, tag="psc")
                        for ci in range(CT):
                            nc.tensor.matmul(
                                s_ps[:, :nw],
                                Wt[:, ci, b * H:(b + 1) * H],
                                xb[:, ci, no:no + nw],
                                start=(ci == 0), stop=(ci == CT - 1))
                        pss.append(s_ps)
                    return pss

                def emit_v():
                    # v_b[t, o], t-major (no bias: folded into proj bias);
                    # psum evacuation split between ACT (512) and DVE (256)
                    vt = kvp.tile([128, NTT, C], BF16, tag="vt")
                    for ho, hw in C_HALVES:
                        for ti, (to, tw) in enumerate(T_TILES):
                            pv = ps.tile([128, 512], F32, tag="pkv")
                            for ci in range(CT):
                                nc.tensor.matmul(
                                    pv[:tw, :hw],
                                    xb[:, ci, to:to + tw],
                                    wv[:, ci, ho:ho + hw],
                                    start=(ci == 0), stop=(ci == CT - 1))
                            if hw == 512:
                                nc.scalar.copy(
                                    vt[:tw, ti, ho:ho + hw], pv[:tw, :hw])
                            else:
                                nc.vector.tensor_copy(
                                    vt[:tw, ti, ho:ho + hw], pv[:tw, :hw])
                    return vt

                # batch 0: Wt isn't ready until the wk2 DMA lands, so run v
                # first; later batches run scores first so the softmax chain
                # overlaps the v matmuls.
                if b == 0:
                    vt = emit_v()
                    emit_q_wt()
                    pss = emit_scores()
                else:
                    pss = emit_scores()
                    vt = emit_v()

                m1 = sm.tile([H, 1], F32, tag="m1")
                m2 = sm.tile([H, 1], F32, tag="m2")
                nc.vector.reduce_max(m1[:], pss[0][:, :N_HALVES[0][1]], axis=AX.X)
                nc.vector.reduce_max(m2[:], pss[1][:, :N_HALVES[1][1]], axis=AX.X)
                am = sm.tile([H, 1], F32, tag="am")
                nc.vector.tensor_max(am[:], m1[:], m2[:])
                negm = sm.tile([H, 1], F32, tag="negm")
                nc.scalar.mul(negm[:], am[:], -1.0)

                e_bf = sm.tile([H, N], BF16, tag="e_bf")
                sums = []
                for i, (no, nw) in enumerate(N_HALVES):
                    acc = sm.tile([H, 1], F32, tag=f"acc{i}")
                    nc.scalar.activation(
                        e_bf[:, no:no + nw], pss[i][:, :nw], AF.Exp,
                        bias=negm[:], scale=1.0, accum_out=acc[:])
                    sums.append(acc)
                ssum = sm.tile([H, 1], F32, tag="ssum")
                nc.vector.tensor_add(ssum[:], sums[0][:], sums[1][:])
                rden = sm.tile([H, 1], F32, tag="rden")
                nc.vector.reciprocal(rden[:], ssum[:])

                # C5: transpose p -> pT[t, h]
                pT = sm.tile([128, NTT, H], BF16, tag="pT")
                for ti, (to, tw) in enumerate(T_TILES):
                    ppt = ps1.tile([128, H], BF16, tag="ptrb")
                    nc.tensor.transpose(
                        ppt[:tw, :], e_bf[:, to:to + tw], identb[:H, :H])
                    nc.vector.tensor_copy(pT[:tw, ti, :], ppt[:tw, :])

                # C6: o_b[h, o] = pT.T @ v  (full 12x768, diag blocks wanted)
                o_tmp = sm.tile([H, C], BF16, tag="o_tmp")
                for ho, hw in C_HALVES:
                    po = ps.tile([H, 512], F32, tag="pav")
                    for ti, (to, tw) in enumerate(T_TILES):
                        nc.tensor.matmul(
                            po[:, :hw],
                            pT[:tw, ti, :],
                            vt[:tw, ti, ho:ho + hw],
                            start=(ti == 0), stop=(ti == NTT - 1))
                    # normalize by 1/sum while evacuating psum
                    nc.vector.tensor_scalar_mul(
                        o_tmp[:, ho:ho + hw], po[:, :hw], rden[:])

                # C7: extract diagonal blocks, transposed: oT_vec[o, b]
                for ci in range(CT):
                    pot = ps1.tile([128, H], BF16, tag="ptrb")
                    nc.tensor.transpose(
                        pot[:, :], o_tmp[:, ci * 128:(ci + 1) * 128],
                        identb[:H, :H])
                    for j in range(2):
                        h = 2 * ci + j
                        nc.vector.tensor_copy(
                            oT_vec[j * 64:(j + 1) * 64, ci, b:b + 1],
                            pot[j * 64:(j + 1) * 64, h:h + 1])

            # ---- C8: proj for all 8 batches ----
            cls_sb = const.tile([BPC, C], F32, tag="cls_sb")
            for ho, hw in C_HALVES:
                pc = ps.tile([BPC, 512], F32, tag="pav")
                for ci in range(CT):
                    nc.tensor.matmul(
                        pc[:, :hw], oT_vec[:, ci, :], wp[:, ci, ho:ho + hw],
                        start=(ci == 0), stop=(ci == CT - 1))
                nc.vector.tensor_add(
                    cls_sb[:, ho:ho + hw], pc[:, :hw], pbr[:, ho:ho + hw])
            nc.sync.dma_start(cls_d.ap(), cls_sb[:])

    nc.compile()
    return nc


@functools.lru_cache(maxsize=1)
def _module():
    return build_module()


def make_in_maps(x, qkv_w, qkv_b, proj_w, proj_b):
    x = np.asarray(x, dtype=np.float32)
    qkv_w = np.asarray(qkv_w, dtype=np.float32)
    qkv_b = np.asarray(qkv_b, dtype=np.float32)
    proj_w = np.asarray(proj_w, dtype=np.float32)
    proj_b = np.asarray(proj_b, dtype=np.float32)

    wk2 = np.ascontiguousarray(qkv_w[C:2 * C]).astype(NPBF16)       # [o, c]
    wv = np.ascontiguousarray(qkv_w[2 * C:].T).astype(NPBF16)       # [c, o]
    wq = np.ascontiguousarray(qkv_w[:C].T * SCALE).astype(NPBF16)   # [c, o]
    wp = np.ascontiguousarray(proj_w.T).astype(NPBF16)              # [c, o]
    qb = np.tile(qkv_b[:C] * SCALE, (BPC, 1)).astype(np.float32)    # [8, C]
    # v bias contributes exactly (vb @ proj_w.T) to cls; fold into proj bias
    pb_eff = proj_b + qkv_b[2 * C:] @ proj_w.T
    pb = np.tile(pb_eff, (BPC, 1)).astype(np.float32)               # [8, C]

    in_maps = []
    for i in range(NCORES):
        xs = x[i * BPC:(i + 1) * BPC]                               # [8, N, C]
        xT = np.ascontiguousarray(xs.transpose(2, 0, 1)).astype(NPBF16)
        xcls = np.ascontiguousarray(xs[:, 0, :].T).astype(NPBF16)   # [C, 8]
        in_maps.append({
            "xT": xT, "wk2": wk2, "wv": wv, "wq": wq, "wp": wp,
            "xcls": xcls, "qb": qb, "pb": pb,
        })
    return in_maps


def kernel(x, qkv_w, qkv_b, proj_w, proj_b):
    nc = _module()
    in_maps = make_in_maps(x, qkv_w, qkv_b, proj_w, proj_b)
    res = bass_utils.run_bass_kernel_spmd(
        nc, in_maps, core_ids=list(range(NCORES)))
    out = np.array(np.asarray(x), dtype=np.float32, copy=True)
    for i in range(NCORES):
        out[i * BPC:(i + 1) * BPC, 0, :] = res.results[i]["cls"]
    return out


# revision 9
# speedup vs baseline: 1.9191x; 1.1643x over previous
"""ClassAttention kernel for 8x TRN2 NeuronCores.

Reference computation (per batch element):
    qkv = x @ qkv_w.T + qkv_b                      # [N, 3C]
    q, k, v = split(qkv)                           # heads H=12, D=64
    s = softmax((q_cls . k) / sqrt(D))             # class-token query only
    cls = (s @ v) @ proj_w.T + proj_b              # [1, C]
    out = concat([cls, x[1:]])                     # rows 1..N pass through

Only the class token row changes, so the device kernel computes just the
[B, C] cls output; rows 1..N are passed through on the host.

Sharding: data-parallel over batch, 8 batches per core, no collectives.
Compute dtype bf16 (fp32 PSUM accumulation), fp32 output.

Key algebraic restructuring (exploits the single class-token query):
  - scores fold the k-projection into a per-batch effective query in
    x-space:  s[b,h,n] = sum_c Wt[c, b*12+h] x[b,n,c]  with
    Wt = wk.T @ blockdiag(q) computed ONCE for all batches (768x96).
    No k vector is ever materialized.
  - the v-projection commutes with the attention average:
    o = p.T @ (x @ wv.T) = (p.T @ x) @ wv.T, so we compute the
    attention-weighted average of x (Z = p.T @ x, 12x768 per batch)
    and project it through wv once per batch. No v vector is ever
    materialized. This collapses the dominant [577x768]x[768x768]
    per-batch projection into [12x768]x[768x768].
  - k bias shifts every score of a head by the same constant ->
    cancels in softmax; dropped.
  - v bias contributes sum_n(p)=1 times vb to the attention output ->
    folds into the proj bias on the host: pb_eff = proj_b + vb @ proj_w.T.
  - softmax normalization (1/sum) is applied while evacuating the
    attention-output psum.

Per-core dataflow (b = 0..8 batches, C=768 in 6 chunks of 128):
  xT[c, b, n], x2[b, n, c]   both layouts of x, host-cast bf16
  qc[b, o]    = wqT.T @ xcls + qb    (wq,qb pre-scaled by 1/8 on host)
  Qblk[o, bh] = block-diagonal q     (PE transpose + aligned copies)
  Wt[c, bh]   = wk2.T @ Qblk         (36 matmuls, once)
  s_b[h, n]   = Wt_b.T @ xT_b
  p_b         = exp(s - max)          (unnormalized, bf16)
  Z_b[h, c]   = pT_b.T @ x2_b
  o_b[h, o]   = ZT_b.T @ wvT          (12x768; diag blocks are the result)
  oT_vec[o,b] = r * diag-extract      (PE transpose + aligned copies)
  cls[b, :]   = oT.T @ wpT + pb_eff

Scores for batch b+1 are emitted ahead of batch b's softmax-dependent
work so the PE never waits on the DVE/ACT softmax chain.
"""

import functools

import numpy as np
import ml_dtypes

import concourse.bass as bass
import concourse.tile as tile
from concourse import bacc, mybir, masks
from concourse import bass_utils

BF16 = mybir.dt.bfloat16
F32 = mybir.dt.float32
NPBF16 = ml_dtypes.bfloat16

B, N, C = 64, 577, 768
H, D = 12, 64
NCORES = 8
BPC = B // NCORES          # 8 batches per core
CT = C // 128              # 6 chunks of the feature dim
SCALE = D ** -0.5          # folded into wq/qb on the host

# token splits: matmul free dim (<=512 fp32 psum bank), K-chunks (<=128)
N_HALVES = [(0, 289), (289, 288)]
C_HALVES = [(0, 512), (512, 256)]
T_TILES = [(0, 128), (128, 128), (256, 128), (384, 128), (512, 65)]
NTT = len(T_TILES)


def build_module():
    nc = bacc.Bacc("TRN2", target_bir_lowering=False, debug=False)

    xT_d = nc.dram_tensor("xT", [C, BPC, N], BF16, kind="ExternalInput")
    x2_d = nc.dram_tensor("x2", [BPC, N, C], BF16, kind="ExternalInput")
    wk2_d = nc.dram_tensor("wk2", [C, C], BF16, kind="ExternalInput")  # [o, c]
    wv_d = nc.dram_tensor("wv", [C, C], BF16, kind="ExternalInput")    # [c, o]
    wq_d = nc.dram_tensor("wq", [C, C], BF16, kind="ExternalInput")    # [c, o]
    wp_d = nc.dram_tensor("wp", [C, C], BF16, kind="ExternalInput")    # [c, o]
    xcls_d = nc.dram_tensor("xcls", [C, BPC], BF16, kind="ExternalInput")
    qb_d = nc.dram_tensor("qb", [BPC, C], F32, kind="ExternalInput")
    pb_d = nc.dram_tensor("pb", [BPC, C], F32, kind="ExternalInput")
    cls_d = nc.dram_tensor("cls", [BPC, C], F32, kind="ExternalOutput")

    AF = mybir.ActivationFunctionType
    AX = mybir.AxisListType

    with tile.TileContext(nc) as tc:
        with (
            tc.tile_pool(name="const", bufs=1) as const,
            tc.tile_pool(name="xp", bufs=3) as xp,
            tc.tile_pool(name="x2p", bufs=2) as x2p,
            tc.tile_pool(name="sm", bufs=2) as sm,
            tc.tile_pool(name="ps", bufs=2, space="PSUM") as ps,
            tc.tile_pool(name="ps1", bufs=1, space="PSUM") as ps1,
        ):
            # ---- DMAs, in the order the pipeline consumes them ----
            # (the cost model serializes dma_starts on one channel)
            xcls = const.tile([128, CT, BPC], BF16, tag="xcls")
            nc.sync.dma_start(
                xcls[:], xcls_d.ap().rearrange("(a p) b -> p a b", p=128))
            wq = const.tile([128, CT, C], BF16, tag="wq")
            wqr = wq_d.ap().rearrange("(a p) o -> p a o", p=128)
            for ci in range(CT):
                nc.sync.dma_start(wq[:, ci, :], wqr[:, ci, :])
            qbr = const.tile([BPC, C], F32, tag="qbr")
            nc.sync.dma_start(qbr[:], qb_d.ap())
            wk2 = const.tile([128, CT, C], BF16, tag="wk2")
            wk2r = wk2_d.ap().rearrange("(a p) o -> p a o", p=128)
            for ci in range(CT):
                nc.sync.dma_start(wk2[:, ci, :], wk2r[:, ci, :])

            xbs = {}

            def load_xb(b):
                xb = xp.tile([128, CT, N], BF16, tag="xb")
                nc.sync.dma_start(
                    xb[:],
                    xT_d.ap()[:, b, :].rearrange("(a p) t -> p a t", p=128))
                xbs[b] = xb

            x2s = {}

            def load_x2(b):
                x2 = x2p.tile([128, NTT, C], BF16, tag="x2")
                for ti, (to, tw) in enumerate(T_TILES):
                    nc.sync.dma_start(
                        x2[:tw, ti, :], x2_d.ap()[b, to:to + tw, :])
                x2s[b] = x2

            load_xb(0)
            load_x2(0)
            wv = const.tile([128, CT, C], BF16, tag="wv")
            wvr = wv_d.ap().rearrange("(a p) o -> p a o", p=128)
            for ho, hw in C_HALVES:
                nc.sync.dma_start(wv[:, :, ho:ho + hw], wvr[:, :, ho:ho + hw])
            load_xb(1)
            load_x2(1)
            pbr = const.tile([BPC, C], F32, tag="pbr")
            nc.sync.dma_start(pbr[:], pb_d.ap())
            wp = const.tile([128, CT, C], BF16, tag="wp")
            nc.sync.dma_start(
                wp[:], wp_d.ap().rearrange("(a p) o -> p a o", p=128))

            identb = const.tile([12, 12], BF16, tag="identb")
            masks.make_identity(nc, identb[:])
            identf = const.tile([BPC, BPC], F32, tag="identf")
            masks.make_identity(nc, identf[:])

            Qblk = const.tile([128, CT, BPC * H], BF16, tag="Qblk")
            nc.vector.memset(Qblk[:], 0.0)
            Wt = const.tile([128, CT, BPC * H], BF16, tag="Wt")
            oT_vec = const.tile([128, CT, BPC], BF16, tag="oT_vec")
            q_sb = const.tile([BPC, C], F32, tag="q_sb")

            # ---- q for all 8 batches, Qblk, Wt ----
            for ho, hw in C_HALVES:
                pq = ps.tile([BPC, 512], F32, tag="pav")
                for ci in range(CT):
                    nc.tensor.matmul(
                        pq[:, :hw], xcls[:, ci, :], wq[:, ci, ho:ho + hw],
                        start=(ci == 0), stop=(ci == CT - 1))
                nc.vector.tensor_add(
                    q_sb[:, ho:ho + hw], pq[:, :hw], qbr[:, ho:ho + hw])

            QblkV = Qblk[:].rearrange("p a (b h) -> p a b h", h=H)
            for ci in range(CT):
                pqt = ps1.tile([128, BPC], F32, tag="pwt")
                nc.tensor.transpose(
                    pqt[:, :], q_sb[:, ci * 128:(ci + 1) * 128],
                    identf[:BPC, :BPC])
                for j in range(2):
                    h = 2 * ci + j
                    nc.vector.tensor_copy(
                        QblkV[j * 64:(j + 1) * 64, ci, :, h],
                        pqt[j * 64:(j + 1) * 64, :])

            for cj in range(CT):
                pw = ps1.tile([128, BPC * H], F32, tag="pwt")
                for oj in range(CT):
                    nc.tensor.matmul(
                        pw[:, :], wk2[:, oj, cj * 128:(cj + 1) * 128],
                        Qblk[:, oj, :],
                        start=(oj == 0), stop=(oj == CT - 1))
                nc.vector.tensor_copy(Wt[:, cj, :], pw[:, :])

            # ---- per-batch pipeline ----
            def emit_scores(b):
                xb = xbs.pop(b)
                pss = []
                for no, nw in N_HALVES:
                    s_ps = ps.tile([H, 512], F32, tag="psc")
                    for ci in range(CT):
                        nc.tensor.matmul(
                            s_ps[:, :nw],
                            Wt[:, ci, b * H:(b + 1) * H],
                            xb[:, ci, no:no + nw],
                            start=(ci == 0), stop=(ci == CT - 1))
                    pss.append(s_ps)
                return pss

            pss = emit_scores(0)
            for b in range(BPC):
                if b + 2 < BPC:
                    load_xb(b + 2)
                    load_x2(b + 2)

                # softmax for b (DVE/ACT)
                m1 = sm.tile([H, 1], F32, tag="m1")
                m2 = sm.tile([H, 1], F32, tag="m2")
                nc.vector.reduce_max(m1[:], pss[0][:, :N_HALVES[0][1]], axis=AX.X)
                nc.vector.reduce_max(m2[:], pss[1][:, :N_HALVES[1][1]], axis=AX.X)
                am = sm.tile([H, 1], F32, tag="am")
                nc.vector.tensor_max(am[:], m1[:], m2[:])
                negm = sm.tile([H, 1], F32, tag="negm")
                nc.scalar.mul(negm[:], am[:], -1.0)

                e_bf = sm.tile([H, N], BF16, tag="e_bf")
                sums = []
                for i, (no, nw) in enumerate(N_HALVES):
                    acc = sm.tile([H, 1], F32, tag=f"acc{i}")
                    nc.scalar.activation(
                        e_bf[:, no:no + nw], pss[i][:, :nw], AF.Exp,
                        bias=negm[:], scale=1.0, accum_out=acc[:])
                    sums.append(acc)
                ssum = sm.tile([H, 1], F32, tag="ssum")
                nc.vector.tensor_add(ssum[:], sums[0][:], sums[1][:])
                rden = sm.tile([H, 1], F32, tag="rden")
                nc.vector.reciprocal(rden[:], ssum[:])

                # scores for b+1 keep the PE busy while b's softmax runs
                if b + 1 < BPC:
                    pss = emit_scores(b + 1)

                # transpose p -> pT[t, h]
                pT = sm.tile([128, NTT, H], BF16, tag="pT")
                for ti, (to, tw) in enumerate(T_TILES):
                    ppt = ps1.tile([128, H], BF16, tag="ptrb")
                    nc.tensor.transpose(
                        ppt[:tw, :], e_bf[:, to:to + tw], identb[:H, :H])
                    nc.vector.tensor_copy(pT[:tw, ti, :], ppt[:tw, :])

                # Z[h, c] = pT.T @ x2  (attention-weighted average of x)
                x2 = x2s.pop(b)
                z_sb = sm.tile([H, C], BF16, tag="z_sb")
                for ho, hw in C_HALVES:
                    pz = ps.tile([H, 512], F32, tag="pz")
                    for ti, (to, tw) in enumerate(T_TILES):
                        nc.tensor.matmul(
                            pz[:, :hw],
                            pT[:tw, ti, :],
                            x2[:tw, ti, ho:ho + hw],
                            start=(ti == 0), stop=(ti == NTT - 1))
                    nc.scalar.copy(z_sb[:, ho:ho + hw], pz[:, :hw])

                # ZT via PE transpose
                zT = sm.tile([128, CT, H], BF16, tag="zT")
                for ci in range(CT):
                    pzt = ps1.tile([128, H], BF16, tag="ptrb")
                    nc.tensor.transpose(
                        pzt[:, :], z_sb[:, ci * 128:(ci + 1) * 128],
                        identb[:H, :H])
                    nc.vector.tensor_copy(zT[:, ci, :], pzt[:, :])

                # o[h, o'] = ZT.T @ wv  (12x768, diag blocks wanted)
                o_tmp = sm.tile([H, C], BF16, tag="o_tmp")
                for ho, hw in C_HALVES:
                    po = ps.tile([H, 512], F32, tag="pav")
                    for ci in range(CT):
                        nc.tensor.matmul(
                            po[:, :hw],
                            zT[:, ci, :],
                            wv[:, ci, ho:ho + hw],
                            start=(ci == 0), stop=(ci == CT - 1))
                    # normalize by 1/sum while evacuating psum
                    nc.vector.tensor_scalar_mul(
                        o_tmp[:, ho:ho + hw], po[:, :hw], rden[:])

                # extract diagonal blocks, transposed: oT_vec[o, b]
                for ci in range(CT):
                    pot = ps1.tile([128, H], BF16, tag="ptrb")
                    nc.tensor.transpose(
                        pot[:, :], o_tmp[:, ci * 128:(ci + 1) * 128],
                        identb[:H, :H])
                    for j in range(2):
                        h = 2 * ci + j
                        nc.vector.tensor_copy(
                            oT_vec[j * 64:(j + 1) * 64, ci, b:b + 1],
                            pot[j * 64:(j + 1) * 64, h:h + 1])

            # ---- proj for all 8 batches ----
            cls_sb = const.tile([BPC, C], F32, tag="cls_sb")
            for ho, hw in C_HALVES:
                pc = ps.tile([BPC, 512], F32, tag="pav")
                for ci in range(CT):
                    nc.tensor.matmul(
                        pc[:, :hw], oT_vec[:, ci, :], wp[:, ci, ho:ho + hw],
                        start=(ci == 0), stop=(ci == CT - 1))
                nc.vector.tensor_add(
                    cls_sb[:, ho:ho + hw], pc[:, :hw], pbr[:, ho:ho + hw])
            nc.sync.dma_start(cls_d.ap(), cls_sb[:])

    nc.compile()
    return nc


@functools.lru_cache(maxsize=1)
def _module():
    return build_module()


def make_in_maps(x, qkv_w, qkv_b, proj_w, proj_b):
    x = np.asarray(x, dtype=np.float32)
    qkv_w = np.asarray(qkv_w, dtype=np.float32)
    qkv_b = np.asarray(qkv_b, dtype=np.float32)
    proj_w = np.asarray(proj_w, dtype=np.float32)
    proj_b = np.asarray(proj_b, dtype=np.float32)

    wk2 = np.ascontiguousarray(qkv_w[C:2 * C]).astype(NPBF16)       # [o, c]
    wv = np.ascontiguousarray(qkv_w[2 * C:].T).astype(NPBF16)       # [c, o]
    wq = np.ascontiguousarray(qkv_w[:C].T * SCALE).astype(NPBF16)   # [c, o]
    wp = np.ascontiguousarray(proj_w.T).astype(NPBF16)              # [c, o]
    qb = np.tile(qkv_b[:C] * SCALE, (BPC, 1)).astype(np.float32)    # [8, C]
    # v bias contributes exactly (vb @ proj_w.T) to cls; fold into proj bias
    pb_eff = proj_b + qkv_b[2 * C:] @ proj_w.T
    pb = np.tile(pb_eff, (BPC, 1)).astype(np.float32)               # [8, C]

    in_maps = []
    for i in range(NCORES):
        xs = x[i * BPC:(i + 1) * BPC]                               # [8, N, C]
        x2 = xs.astype(NPBF16)                                      # [8, N, C]
        xT = np.ascontiguousarray(xs.transpose(2, 0, 1)).astype(NPBF16)
        xcls = np.ascontiguousarray(xs[:, 0, :].T).astype(NPBF16)   # [C, 8]
        in_maps.append({
            "xT": xT, "x2": x2, "wk2": wk2, "wv": wv, "wq": wq, "wp": wp,
            "xcls": xcls, "qb": qb, "pb": pb,
        })
    return in_maps


def kernel(x, qkv_w, qkv_b, proj_w, proj_b):
    nc = _module()
    in_maps = make_in_maps(x, qkv_w, qkv_b, proj_w, proj_b)
    res = bass_utils.run_bass_kernel_spmd(
        nc, in_maps, core_ids=list(range(NCORES)))
    out = np.array(np.asarray(x), dtype=np.float32, copy=True)
    for i in range(NCORES):
        out[i * BPC:(i + 1) * BPC, 0, :] = res.results[i]["cls"]
    return out


# revision 12
# speedup vs baseline: 1.9838x; 1.0337x over previous
"""ClassAttention kernel for 8x TRN2 NeuronCores.

Reference computation (per batch element):
    qkv = x @ qkv_w.T + qkv_b                      # [N, 3C]
    q, k, v = split(qkv)                           # heads H=12, D=64
    s = softmax((q_cls . k) / sqrt(D))             # class-token query only
    cls = (s @ v) @ proj_w.T + proj_b              # [1, C]
    out = concat([cls, x[1:]])                     # rows 1..N pass through

Only the class token row changes, so the device kernel computes just the
[B, C] cls output; rows 1..N are passed through on the host.

Sharding: data-parallel over batch, 8 batches per core, no collectives.
Compute dtype bf16 (fp32 PSUM accumulation), fp32 output.

Key algebraic restructuring (exploits the single class-token query):
  - scores fold the k-projection into a per-batch effective query in
    x-space:  s[b,h,n] = sum_c Wt[c, b*12+h] x[b,n,c]  with
    Wt = wk.T @ blockdiag(q) computed ONCE for all batches (768x96).
    No k vector is ever materialized.
  - the v-projection commutes with the attention average:
    o = p.T @ (x @ wv.T) = (p.T @ x) @ wv.T, so we compute the
    attention-weighted average of x (Z = p.T @ x, 12x768 per batch)
    and project it through wv once per batch. No v vector is ever
    materialized. This collapses the dominant [577x768]x[768x768]
    per-batch projection into [12x768]x[768x768].
  - k bias shifts every score of a head by the same constant ->
    cancels in softmax; dropped.
  - v bias contributes sum_n(p)=1 times vb to the attention output ->
    folds into the proj bias on the host: pb_eff = proj_b + vb @ proj_w.T.
  - softmax normalization (1/sum) is applied while evacuating the
    attention-output psum.

Per-core dataflow (b = 0..8 batches, C=768 in 6 chunks of 128):
  xT[c, b, n], x2[b, n, c]   both layouts of x, host-cast bf16
  qc[b, o]    = wqT.T @ xcls + qb    (wq,qb pre-scaled by 1/8 on host)
  Qblk[o, bh] = block-diagonal q     (PE transpose + aligned copies)
  Wt[c, bh]   = wk2.T @ Qblk         (36 matmuls, once)
  s_b[h, n]   = Wt_b.T @ xT_b
  p_b         = exp(s - max)          (unnormalized, bf16)
  Z_b[h, c]   = pT_b.T @ x2_b
  o_b[h, o]   = ZT_b.T @ wvT          (12x768; diag blocks are the result)
  oT_vec[o,b] = r * diag-extract      (PE transpose + aligned copies)
  cls[b, :]   = oT.T @ wpT + pb_eff

Scores for batch b+1 are emitted ahead of batch b's softmax-dependent
work so the PE never waits on the DVE/ACT softmax chain.
"""

import functools

import numpy as np
import ml_dtypes

import concourse.bass as bass
import concourse.tile as tile
from concourse import bacc, mybir, masks
from concourse import bass_utils

BF16 = mybir.dt.bfloat16
F32 = mybir.dt.float32
NPBF16 = ml_dtypes.bfloat16

B, N, C = 64, 577, 768
H, D = 12, 64
NCORES = 8
BPC = B // NCORES          # 8 batches per core
CT = C // 128              # 6 chunks of the feature dim
SCALE = D ** -0.5          # folded into wq/qb on the host

# token splits: matmul free dim (<=512 fp32 psum bank), K-chunks (<=128)
N_HALVES = [(0, 289), (289, 288)]
C_HALVES = [(0, 512), (512, 256)]
T_TILES = [(0, 128), (128, 128), (256, 128), (384, 128), (512, 65)]
NTT = len(T_TILES)


def build_module():
    nc = bacc.Bacc("TRN2", target_bir_lowering=False, debug=False)

    xT_d = nc.dram_tensor("xT", [C, BPC, N], BF16, kind="ExternalInput")
    x2_d = nc.dram_tensor("x2", [BPC, N, C], BF16, kind="ExternalInput")
    wk2_d = nc.dram_tensor("wk2", [C, C], BF16, kind="ExternalInput")  # [o, c]
    wv_d = nc.dram_tensor("wv", [C, C], BF16, kind="ExternalInput")    # [c, o]
    wq_d = nc.dram_tensor("wq", [C, C], BF16, kind="ExternalInput")    # [c, o]
    wp_d = nc.dram_tensor("wp", [C, C], BF16, kind="ExternalInput")    # [c, o]
    xcls_d = nc.dram_tensor("xcls", [C, BPC], BF16, kind="ExternalInput")
    qb_d = nc.dram_tensor("qb", [BPC, C], F32, kind="ExternalInput")
    pb_d = nc.dram_tensor("pb", [BPC, C], F32, kind="ExternalInput")
    cls_d = nc.dram_tensor("cls", [BPC, C], F32, kind="ExternalOutput")

    AF = mybir.ActivationFunctionType
    AX = mybir.AxisListType

    with tile.TileContext(nc) as tc:
        with (
            tc.tile_pool(name="const", bufs=1) as const,
            tc.tile_pool(name="xp", bufs=4) as xp,
            tc.tile_pool(name="x2p", bufs=4) as x2p,
            tc.tile_pool(name="sm", bufs=3) as sm,
            tc.tile_pool(name="ps", bufs=2, space="PSUM") as ps,
            tc.tile_pool(name="ps1", bufs=1, space="PSUM") as ps1,
        ):
            # ---- DMAs, in the order the pipeline consumes them ----
            # (the cost model serializes dma_starts on one channel)
            xcls = const.tile([128, CT, BPC], BF16, tag="xcls")
            nc.sync.dma_start(
                xcls[:], xcls_d.ap().rearrange("(a p) b -> p a b", p=128))
            wq = const.tile([128, CT, C], BF16, tag="wq")
            wqr = wq_d.ap().rearrange("(a p) o -> p a o", p=128)
            for ci in range(CT):
                nc.sync.dma_start(wq[:, ci, :], wqr[:, ci, :])
            qbr = const.tile([BPC, C], F32, tag="qbr")
            nc.sync.dma_start(qbr[:], qb_d.ap())
            wk2 = const.tile([128, CT, C], BF16, tag="wk2")
            wk2r = wk2_d.ap().rearrange("(a p) o -> p a o", p=128)
            for ci in range(CT):
                nc.sync.dma_start(wk2[:, ci, :], wk2r[:, ci, :])

            xbs = {}

            def load_xb(b):
                xb = xp.tile([128, CT, N], BF16, tag="xb")
                nc.sync.dma_start(
                    xb[:],
                    xT_d.ap()[:, b, :].rearrange("(a p) t -> p a t", p=128))
                xbs[b] = xb

            x2s = {}

            def load_x2(b):
                x2 = x2p.tile([128, NTT, C], BF16, tag="x2")
                for ti, (to, tw) in enumerate(T_TILES):
                    nc.sync.dma_start(
                        x2[:tw, ti, :], x2_d.ap()[b, to:to + tw, :])
                x2s[b] = x2

            load_xb(0)
            load_x2(0)
            wv = const.tile([128, CT, C], BF16, tag="wv")
            wvr = wv_d.ap().rearrange("(a p) o -> p a o", p=128)
            nc.sync.dma_start(wv[:, :, 0:512], wvr[:, :, 0:512])
            load_xb(1)
            nc.sync.dma_start(wv[:, :, 512:768], wvr[:, :, 512:768])
            load_x2(1)
            load_xb(2)
            load_x2(2)
            pbr = const.tile([BPC, C], F32, tag="pbr")
            nc.sync.dma_start(pbr[:], pb_d.ap())
            wp = const.tile([128, CT, C], BF16, tag="wp")
            nc.sync.dma_start(
                wp[:], wp_d.ap().rearrange("(a p) o -> p a o", p=128))

            identb = const.tile([12, 12], BF16, tag="identb")
            masks.make_identity(nc, identb[:])
            identf = const.tile([BPC, BPC], F32, tag="identf")
            masks.make_identity(nc, identf[:])

            Qblk = const.tile([128, CT, BPC * H], BF16, tag="Qblk")
            nc.vector.memset(Qblk[:], 0.0)
            Wt = const.tile([128, CT, BPC * H], BF16, tag="Wt")
            oT_vec = const.tile([128, CT, BPC], BF16, tag="oT_vec")
            q_sb = const.tile([BPC, C], F32, tag="q_sb")

            # ---- q for all 8 batches, Qblk, Wt ----
            for ho, hw in C_HALVES:
                pq = ps.tile([BPC, 512], F32, tag="pav")
                for ci in range(CT):
                    nc.tensor.matmul(
                        pq[:, :hw], xcls[:, ci, :], wq[:, ci, ho:ho + hw],
                        start=(ci == 0), stop=(ci == CT - 1))
                nc.vector.tensor_add(
                    q_sb[:, ho:ho + hw], pq[:, :hw], qbr[:, ho:ho + hw])

            QblkV = Qblk[:].rearrange("p a (b h) -> p a b h", h=H)
            for ci in range(CT):
                pqt = ps1.tile([128, BPC], F32, tag="pwt")
                nc.tensor.transpose(
                    pqt[:, :], q_sb[:, ci * 128:(ci + 1) * 128],
                    identf[:BPC, :BPC])
                for j in range(2):
                    h = 2 * ci + j
                    nc.vector.tensor_copy(
                        QblkV[j * 64:(j + 1) * 64, ci, :, h],
                        pqt[j * 64:(j + 1) * 64, :])

            for cj in range(CT):
                pw = ps1.tile([128, BPC * H], F32, tag="pwt")
                for oj in range(CT):
                    nc.tensor.matmul(
                        pw[:, :], wk2[:, oj, cj * 128:(cj + 1) * 128],
                        Qblk[:, oj, :],
                        start=(oj == 0), stop=(oj == CT - 1))
                nc.vector.tensor_copy(Wt[:, cj, :], pw[:, :])

            # ---- per-batch software pipeline ----
            # Emission (= PE execution) order per slot k:
            #   A(k+2) scores | S(k+2) softmax (DVE/ACT) | X(k-1) extract |
            #   P(k) pT | Z(k) | T(k) ZT | O(k) attn-out
            # so every DVE/ACT round-trip is covered by PE work from a
            # neighboring slot. Psum evacuations are interleaved inside the
            # Z/O chains (half0 evacuates while half1's matmuls stream).
            st = {}

            def emit_A(b):
                xb = xbs.pop(b)
                pss = []
                for no, nw in N_HALVES:
                    s_ps = ps.tile([H, 512], F32, tag="pscz")
                    for ci in range(CT):
                        nc.tensor.matmul(
                            s_ps[:, :nw],
                            Wt[:, ci, b * H:(b + 1) * H],
                            xb[:, ci, no:no + nw],
                            start=(ci == 0), stop=(ci == CT - 1))
                    pss.append(s_ps)
                st[b] = {"pss": pss}

            def emit_S(b):
                s = st[b]
                pss = s["pss"]
                m1 = sm.tile([H, 1], F32, tag="m1")
                m2 = sm.tile([H, 1], F32, tag="m2")
                nc.vector.reduce_max(m1[:], pss[0][:, :N_HALVES[0][1]],
                                     axis=AX.X)
                nc.vector.reduce_max(m2[:], pss[1][:, :N_HALVES[1][1]],
                                     axis=AX.X)
                am = sm.tile([H, 1], F32, tag="am")
                nc.vector.tensor_max(am[:], m1[:], m2[:])
                negm = sm.tile([H, 1], F32, tag="negm")
                nc.scalar.mul(negm[:], am[:], -1.0)

                e_bf = sm.tile([H, N], BF16, tag="e_bf")
                sums = []
                for i, (no, nw) in enumerate(N_HALVES):
                    acc = sm.tile([H, 1], F32, tag=f"acc{i}")
                    nc.scalar.activation(
                        e_bf[:, no:no + nw], pss[i][:, :nw], AF.Exp,
                        bias=negm[:], scale=1.0, accum_out=acc[:])
                    sums.append(acc)
                ssum = sm.tile([H, 1], F32, tag="ssum")
                nc.vector.tensor_add(ssum[:], sums[0][:], sums[1][:])
                rden = sm.tile([H, 1], F32, tag="rden")
                nc.vector.reciprocal(rden[:], ssum[:])
                s["e_bf"], s["rden"] = e_bf, rden

            def emit_PZ(b):
                s = st[b]
                e_bf = s["e_bf"]
                pT = sm.tile([128, NTT, H], BF16, tag="pT")
                for ti, (to, tw) in enumerate(T_TILES):
                    ppt = ps.tile([128, H], BF16, tag="ptrb")
                    nc.tensor.transpose(
                        ppt[:tw, :], e_bf[:, to:to + tw], identb[:H, :H])
                    nc.scalar.copy(pT[:tw, ti, :], ppt[:tw, :])

                x2 = x2s.pop(b)
                z_sb = sm.tile([H, C], BF16, tag="z_sb")
                for ho, hw in C_HALVES:
                    pz = ps.tile([H, 512], F32, tag="pscz")
                    for ti, (to, tw) in enumerate(T_TILES):
                        nc.tensor.matmul(
                            pz[:, :hw],
                            pT[:tw, ti, :],
                            x2[:tw, ti, ho:ho + hw],
                            start=(ti == 0), stop=(ti == NTT - 1))
                    nc.scalar.copy(z_sb[:, ho:ho + hw], pz[:, :hw])
                s["z_sb"] = z_sb

            def emit_TO(b):
                s = st[b]
                z_sb, rden = s["z_sb"], s["rden"]
                zT = sm.tile([128, CT, H], BF16, tag="zT")
                for ci in range(CT):
                    pzt = ps.tile([128, H], BF16, tag="ptrb")
                    nc.tensor.transpose(
                        pzt[:, :], z_sb[:, ci * 128:(ci + 1) * 128],
                        identb[:H, :H])
                    nc.scalar.copy(zT[:, ci, :], pzt[:, :])

                o_tmp = sm.tile([H, C], BF16, tag="o_tmp")
                for ho, hw in C_HALVES:
                    po = ps.tile([H, 512], F32, tag="pav")
                    for ci in range(CT):
                        nc.tensor.matmul(
                            po[:, :hw],
                            zT[:, ci, :],
                            wv[:, ci, ho:ho + hw],
                            start=(ci == 0), stop=(ci == CT - 1))
                    # normalize by 1/sum while evacuating psum
                    nc.vector.tensor_scalar_mul(
                        o_tmp[:, ho:ho + hw], po[:, :hw], rden[:])
                s["o_tmp"] = o_tmp

            def emit_X(b):
                s = st.pop(b)
                o_tmp = s["o_tmp"]
                for ci in range(CT):
                    pot = ps.tile([128, H], BF16, tag="ptrb")
                    nc.tensor.transpose(
                        pot[:, :], o_tmp[:, ci * 128:(ci + 1) * 128],
                        identb[:H, :H])
                    for j in range(2):
                        h = 2 * ci + j
                        nc.vector.tensor_copy(
                            oT_vec[j * 64:(j + 1) * 64, ci, b:b + 1],
                            pot[j * 64:(j + 1) * 64, h:h + 1])

            emit_A(0)
            emit_S(0)
            emit_A(1)
            emit_S(1)
            for k in range(BPC):
                if k + 3 < BPC:
                    load_xb(k + 3)
                    load_x2(k + 3)
                if k + 2 < BPC:
                    emit_A(k + 2)
                    emit_S(k + 2)
                if k >= 1:
                    emit_X(k - 1)
                emit_PZ(k)
                emit_TO(k)
            emit_X(BPC - 1)

            # ---- proj for all 8 batches ----
            cls_sb = const.tile([BPC, C], F32, tag="cls_sb")
            for ho, hw in C_HALVES:
                pc = ps.tile([BPC, 512], F32, tag="pav")
                for ci in range(CT):
                    nc.tensor.matmul(
                        pc[:, :hw], oT_vec[:, ci, :], wp[:, ci, ho:ho + hw],
                        start=(ci == 0), stop=(ci == CT - 1))
                nc.vector.tensor_add(
                    cls_sb[:, ho:ho + hw], pc[:, :hw], pbr[:, ho:ho + hw])
            nc.sync.dma_start(cls_d.ap(), cls_sb[:])

    nc.compile()
    return nc


@functools.lru_cache(maxsize=1)
def _module():
    return build_module()


def make_in_maps(x, qkv_w, qkv_b, proj_w, proj_b):
    x = np.asarray(x, dtype=np.float32)
    qkv_w = np.asarray(qkv_w, dtype=np.float32)
    qkv_b = np.asarray(qkv_b, dtype=np.float32)
    proj_w = np.asarray(proj_w, dtype=np.float32)
    proj_b = np.asarray(proj_b, dtype=np.float32)

    wk2 = np.ascontiguousarray(qkv_w[C:2 * C]).astype(NPBF16)       # [o, c]
    wv = np.ascontiguousarray(qkv_w[2 * C:].T).astype(NPBF16)       # [c, o]
    wq = np.ascontiguousarray(qkv_w[:C].T * SCALE).astype(NPBF16)   # [c, o]
    wp = np.ascontiguousarray(proj_w.T).astype(NPBF16)              # [c, o]
    qb = np.tile(qkv_b[:C] * SCALE, (BPC, 1)).astype(np.float32)    # [8, C]
    # v bias contributes exactly (vb @ proj_w.T) to cls; fold into proj bias
    pb_eff = proj_b + qkv_b[2 * C:] @ proj_w.T
    pb = np.tile(pb_eff, (BPC, 1)).astype(np.float32)               # [8, C]

    in_maps = []
    for i in range(NCORES):
        xs = x[i * BPC:(i + 1) * BPC]                               # [8, N, C]
        x2 = xs.astype(NPBF16)                                      # [8, N, C]
        xT = np.ascontiguousarray(xs.transpose(2, 0, 1)).astype(NPBF16)
        xcls = np.ascontiguousarray(xs[:, 0, :].T).astype(NPBF16)   # [C, 8]
        in_maps.append({
            "xT": xT, "x2": x2, "wk2": wk2, "wv": wv, "wq": wq, "wp": wp,
            "xcls": xcls, "qb": qb, "pb": pb,
        })
    return in_maps


def kernel(x, qkv_w, qkv_b, proj_w, proj_b):
    nc = _module()
    in_maps = make_in_maps(x, qkv_w, qkv_b, proj_w, proj_b)
    res = bass_utils.run_bass_kernel_spmd(
        nc, in_maps, core_ids=list(range(NCORES)))
    out = np.array(np.asarray(x), dtype=np.float32, copy=True)
    for i in range(NCORES):
        out[i * BPC:(i + 1) * BPC, 0, :] = res.results[i]["cls"]
    return out


# revision 14
# speedup vs baseline: 2.1465x; 1.0821x over previous
"""ClassAttention kernel for 8x TRN2 NeuronCores.

Reference computation (per batch element):
    qkv = x @ qkv_w.T + qkv_b                      # [N, 3C]
    q, k, v = split(qkv)                           # heads H=12, D=64
    s = softmax((q_cls . k) / sqrt(D))             # class-token query only
    cls = (s @ v) @ proj_w.T + proj_b              # [1, C]
    out = concat([cls, x[1:]])                     # rows 1..N pass through

Only the class token row changes, so the device kernel computes just the
[B, C] cls output; rows 1..N are passed through on the host.

Sharding: data-parallel over batch, 8 batches per core, no collectives.
Compute dtype bf16 (fp32 PSUM accumulation), fp32 output.

Key algebraic restructuring (exploits the single class-token query):
  - scores fold the k-projection into a per-batch effective query in
    x-space:  s[b,h,n] = sum_c Wt[c, b*12+h] x[b,n,c]  with
    Wt = wk.T @ blockdiag(q) computed ONCE for all batches (768x96).
    No k vector is ever materialized.
  - the v-projection commutes with the attention average:
    o = p.T @ (x @ wv.T) = (p.T @ x) @ wv.T, so we compute the
    attention-weighted average of x (Z = p.T @ x, 12x768 per batch)
    and project it through wv once per batch. No v vector is ever
    materialized. This collapses the dominant [577x768]x[768x768]
    per-batch projection into [12x768]x[768x768].
  - k bias shifts every score of a head by the same constant ->
    cancels in softmax; dropped.
  - v bias contributes sum_n(p)=1 times vb to the attention output ->
    folds into the proj bias on the host: pb_eff = proj_b + vb @ proj_w.T.
  - softmax normalization (1/sum) is applied while evacuating the
    attention-output psum.

Per-core dataflow (b = 0..8 batches, C=768 in 6 chunks of 128):
  xT[c, b, n], x2[b, n, c]   both layouts of x, host-cast bf16
  qc[b, o]    = wqT.T @ xcls + qb    (wq,qb pre-scaled by 1/8 on host)
  Qblk[o, bh] = block-diagonal q     (PE transpose + aligned copies)
  Wt[c, bh]   = wk2.T @ Qblk         (36 matmuls, once)
  s_b[h, n]   = Wt_b.T @ xT_b
  p_b         = exp(s - max)          (unnormalized, bf16)
  Z_b[h, c]   = pT_b.T @ x2_b
  o_b[h, o]   = ZT_b.T @ wvT          (12x768; diag blocks are the result)
  oT_vec[o,b] = r * diag-extract      (PE transpose + aligned copies)
  cls[b, :]   = oT.T @ wpT + pb_eff

Scores for batch b+1 are emitted ahead of batch b's softmax-dependent
work so the PE never waits on the DVE/ACT softmax chain.
"""

import functools

import numpy as np
import ml_dtypes

import concourse.bass as bass
import concourse.tile as tile
from concourse import bacc, mybir, masks
from concourse import bass_utils

BF16 = mybir.dt.bfloat16
F32 = mybir.dt.float32
NPBF16 = ml_dtypes.bfloat16

B, N, C = 64, 577, 768
H, D = 12, 64
NCORES = 8
BPC = B // NCORES          # 8 batches per core
CT = C // 128              # 6 chunks of the feature dim
SCALE = D ** -0.5          # folded into wq/qb on the host

# token splits: matmul free dim (<=512 fp32 psum bank), K-chunks (<=128)
N_HALVES = [(0, 289), (289, 288)]
C_HALVES = [(0, 512), (512, 256)]
T_TILES = [(0, 128), (128, 128), (256, 128), (384, 128), (512, 65)]
NTT = len(T_TILES)


def build_module():
    nc = bacc.Bacc("TRN2", target_bir_lowering=False, debug=False)

    xT_d = nc.dram_tensor("xT", [C, BPC, N], BF16, kind="ExternalInput")
    x2_d = nc.dram_tensor("x2", [BPC, N, C], BF16, kind="ExternalInput")
    wk2_d = nc.dram_tensor("wk2", [C, C], BF16, kind="ExternalInput")  # [o, c]
    wv_d = nc.dram_tensor("wv", [C, C], BF16, kind="ExternalInput")    # [c, o]
    wq_d = nc.dram_tensor("wq", [C, C], BF16, kind="ExternalInput")    # [c, o]
    wp_d = nc.dram_tensor("wp", [C, C], BF16, kind="ExternalInput")    # [c, o]
    xcls_d = nc.dram_tensor("xcls", [C, BPC], BF16, kind="ExternalInput")
    qb_d = nc.dram_tensor("qb", [BPC, C], F32, kind="ExternalInput")
    pb_d = nc.dram_tensor("pb", [BPC, C], F32, kind="ExternalInput")
    cls_d = nc.dram_tensor("cls", [BPC, C], F32, kind="ExternalOutput")

    AF = mybir.ActivationFunctionType
    AX = mybir.AxisListType

    with tile.TileContext(nc) as tc:
        with (
            tc.tile_pool(name="const", bufs=1) as const,
            tc.tile_pool(name="xp", bufs=4) as xp,
            tc.tile_pool(name="x2p", bufs=4) as x2p,
            tc.tile_pool(name="sm", bufs=3) as sm,
            tc.tile_pool(name="ps", bufs=2, space="PSUM") as ps,
        ):
            # ---- DMAs, in the order the pipeline consumes them ----
            # (the cost model serializes dma_starts on one channel)
            xcls = const.tile([128, CT, BPC], BF16, tag="xcls")
            nc.sync.dma_start(
                xcls[:], xcls_d.ap().rearrange("(a p) b -> p a b", p=128))
            wq = const.tile([128, CT, C], BF16, tag="wq")
            wqr = wq_d.ap().rearrange("(a p) o -> p a o", p=128)
            for ci in range(CT):
                nc.sync.dma_start(wq[:, ci, :], wqr[:, ci, :])
            qbr = const.tile([BPC, C], F32, tag="qbr")
            nc.sync.dma_start(qbr[:], qb_d.ap())
            wk2 = const.tile([128, CT, C], BF16, tag="wk2")
            wk2r = wk2_d.ap().rearrange("(a p) o -> p a o", p=128)
            for ci in range(CT):
                nc.sync.dma_start(wk2[:, ci, :], wk2r[:, ci, :])

            xbs = {}

            def load_xb(b):
                xb = xp.tile([128, CT, N], BF16, tag="xb")
                nc.sync.dma_start(
                    xb[:],
                    xT_d.ap()[:, b, :].rearrange("(a p) t -> p a t", p=128))
                xbs[b] = xb

            x2s = {}

            def load_x2(b):
                x2 = x2p.tile([128, NTT, C], BF16, tag="x2")
                for ti, (to, tw) in enumerate(T_TILES):
                    nc.sync.dma_start(
                        x2[:tw, ti, :], x2_d.ap()[b, to:to + tw, :])
                x2s[b] = x2

            load_xb(0)
            load_x2(0)
            wv = const.tile([128, CT, C], BF16, tag="wv")
            wvr = wv_d.ap().rearrange("(a p) o -> p a o", p=128)
            nc.sync.dma_start(wv[:, :, 0:512], wvr[:, :, 0:512])
            load_xb(1)
            nc.sync.dma_start(wv[:, :, 512:768], wvr[:, :, 512:768])
            load_x2(1)
            load_xb(2)
            load_x2(2)
            pbr = const.tile([BPC, C], F32, tag="pbr")
            nc.sync.dma_start(pbr[:], pb_d.ap())
            wp = const.tile([128, CT, C], BF16, tag="wp")
            nc.sync.dma_start(
                wp[:], wp_d.ap().rearrange("(a p) o -> p a o", p=128))

            identb = const.tile([12, 12], BF16, tag="identb")
            masks.make_identity(nc, identb[:])
            identf = const.tile([BPC, BPC], F32, tag="identf")
            masks.make_identity(nc, identf[:])

            Qblk = const.tile([128, CT, BPC * H], BF16, tag="Qblk")
            nc.vector.memset(Qblk[:], 0.0)
            Wt = const.tile([128, CT, BPC * H], BF16, tag="Wt")
            oT_vec = const.tile([128, CT, BPC], BF16, tag="oT_vec")
            q_sb = const.tile([BPC, C], F32, tag="q_sb")
            # ZT for 4 batches per 128-col group, 32-aligned (zero padding)
            ZT_all = const.tile([128, CT, 2, 128], BF16, tag="ZT_all")
            nc.vector.memset(ZT_all[:], 0.0)
            o_sb = const.tile([128, 2, C], BF16, tag="o_sb")
            # identity replicated at partition bases 0/32/64/96 for the
            # 32-aligned diag-extraction transposes
            ident4 = const.tile([128, H], BF16, tag="ident4")
            for _j in range(4):
                masks.make_identity(nc, ident4[32 * _j:32 * _j + H, :])

            # ---- q for all 8 batches, Qblk, Wt ----
            for ho, hw in C_HALVES:
                pq = ps.tile([BPC, 512], F32, tag="pav")
                for ci in range(CT):
                    nc.tensor.matmul(
                        pq[:, :hw], xcls[:, ci, :], wq[:, ci, ho:ho + hw],
                        start=(ci == 0), stop=(ci == CT - 1))
                nc.vector.tensor_add(
                    q_sb[:, ho:ho + hw], pq[:, :hw], qbr[:, ho:ho + hw])

            QblkV = Qblk[:].rearrange("p a (b h) -> p a b h", h=H)
            for ci in range(CT):
                pqt = ps.tile([128, BPC], F32, tag="pav")
                nc.tensor.transpose(
                    pqt[:, :], q_sb[:, ci * 128:(ci + 1) * 128],
                    identf[:BPC, :BPC])
                for j in range(2):
                    h = 2 * ci + j
                    nc.vector.tensor_copy(
                        QblkV[j * 64:(j + 1) * 64, ci, :, h],
                        pqt[j * 64:(j + 1) * 64, :])

            for cj in range(CT):
                pw = ps.tile([128, BPC * H], F32, tag="pav")
                for oj in range(CT):
                    nc.tensor.matmul(
                        pw[:, :], wk2[:, oj, cj * 128:(cj + 1) * 128],
                        Qblk[:, oj, :],
                        start=(oj == 0), stop=(oj == CT - 1))
                nc.vector.tensor_copy(Wt[:, cj, :], pw[:, :])

            # ---- per-batch software pipeline ----
            # Emission (= PE execution) order per slot k:
            #   A(k+2) scores | S(k+2) softmax (DVE/ACT) | X(k-1) extract |
            #   P(k) pT | Z(k) | T(k) ZT | O(k) attn-out
            # so every DVE/ACT round-trip is covered by PE work from a
            # neighboring slot. Psum evacuations are interleaved inside the
            # Z/O chains (half0 evacuates while half1's matmuls stream).
            st = {}

            def emit_A(b):
                xb = xbs.pop(b)
                pss = []
                for no, nw in N_HALVES:
                    s_ps = ps.tile([H, 512], F32, tag="pscz")
                    for ci in range(CT):
                        nc.tensor.matmul(
                            s_ps[:, :nw],
                            Wt[:, ci, b * H:(b + 1) * H],
                            xb[:, ci, no:no + nw],
                            start=(ci == 0), stop=(ci == CT - 1))
                    pss.append(s_ps)
                st[b] = {"pss": pss}

            def emit_S(b):
                s = st[b]
                pss = s["pss"]
                m1 = sm.tile([H, 1], F32, tag="m1")
                m2 = sm.tile([H, 1], F32, tag="m2")
                nc.vector.reduce_max(m1[:], pss[0][:, :N_HALVES[0][1]],
                                     axis=AX.X)
                nc.vector.reduce_max(m2[:], pss[1][:, :N_HALVES[1][1]],
                                     axis=AX.X)
                am = sm.tile([H, 1], F32, tag="am")
                nc.vector.tensor_max(am[:], m1[:], m2[:])
                negm = sm.tile([H, 1], F32, tag="negm")
                nc.scalar.mul(negm[:], am[:], -1.0)

                e_bf = sm.tile([H, N], BF16, tag="e_bf")
                sums = []
                for i, (no, nw) in enumerate(N_HALVES):
                    acc = sm.tile([H, 1], F32, tag=f"acc{i}")
                    nc.scalar.activation(
                        e_bf[:, no:no + nw], pss[i][:, :nw], AF.Exp,
                        bias=negm[:], scale=1.0, accum_out=acc[:])
                    sums.append(acc)
                ssum = sm.tile([H, 1], F32, tag="ssum")
                nc.vector.tensor_add(ssum[:], sums[0][:], sums[1][:])
                rden = sm.tile([H, 1], F32, tag="rden")
                nc.vector.reciprocal(rden[:], ssum[:])
                # normalized attention weights p = e / sum (bf16)
                e_nm = sm.tile([H, N], BF16, tag="e_nm")
                nc.vector.tensor_scalar_mul(e_nm[:], e_bf[:], rden[:])
                s["e_nm"] = e_nm

            def emit_PZ(b):
                s = st[b]
                e_nm = s["e_nm"]
                pT = sm.tile([128, NTT, H], BF16, tag="pT")
                for ti, (to, tw) in enumerate(T_TILES):
                    ppt = ps.tile([128, H], BF16, tag="ptrb")
                    nc.tensor.transpose(
                        ppt[:tw, :], e_nm[:, to:to + tw], identb[:H, :H])
                    nc.scalar.copy(pT[:tw, ti, :], ppt[:tw, :])

                x2 = x2s.pop(b)
                z_sb = sm.tile([H, C], BF16, tag="z_sb")
                for ho, hw in C_HALVES:
                    pz = ps.tile([H, 512], F32, tag="pscz")
                    for ti, (to, tw) in enumerate(T_TILES):
                        nc.tensor.matmul(
                            pz[:, :hw],
                            pT[:tw, ti, :],
                            x2[:tw, ti, ho:ho + hw],
                            start=(ti == 0), stop=(ti == NTT - 1))
                    nc.scalar.copy(z_sb[:, ho:ho + hw], pz[:, :hw])
                s["z_sb"] = z_sb

            def emit_T(b):
                # ZT for batch b into its 32-aligned column group of ZT_all
                s = st.pop(b)
                z_sb = s["z_sb"]
                g, j = b // 4, b % 4
                for ci in range(CT):
                    pzt = ps.tile([128, H], BF16, tag="ptrb")
                    nc.tensor.transpose(
                        pzt[:, :], z_sb[:, ci * 128:(ci + 1) * 128],
                        identb[:H, :H])
                    nc.scalar.copy(
                        ZT_all[:, ci, g, 32 * j:32 * j + H], pzt[:, :])

            def emit_O(g):
                # o rows for 4 batches at once: psum rows 32j..32j+12 = batch
                # 4g+j  (M-packed; zero columns of ZT_all give zero rows)
                for ho, hw in C_HALVES:
                    po = ps.tile([128, 512], F32, tag="pav")
                    for ci in range(CT):
                        nc.tensor.matmul(
                            po[:, :hw],
                            ZT_all[:, ci, g, :],
                            wv[:, ci, ho:ho + hw],
                            start=(ci == 0), stop=(ci == CT - 1))
                    nc.scalar.copy(o_sb[:, g, ho:ho + hw], po[:, :hw])

            def emit_X(b):
                # extract diagonal blocks of batch b, transposed: oT_vec[o, b]
                g, j = b // 4, b % 4
                for ci in range(CT):
                    pot = ps.tile([128, H], BF16, tag="ptrb")
                    nc.tensor.transpose(
                        pot[:, :],
                        o_sb[32 * j:32 * j + H, g, ci * 128:(ci + 1) * 128],
                        ident4[32 * j:32 * j + H, :],
                        tile_position=(32 * j, 0))
                    for jj in range(2):
                        h = 2 * ci + jj
                        nc.vector.tensor_copy(
                            oT_vec[jj * 64:(jj + 1) * 64, ci, b:b + 1],
                            pot[jj * 64:(jj + 1) * 64, h:h + 1])

            emit_A(0)
            emit_S(0)
            emit_A(1)
            emit_S(1)
            for k in range(BPC):
                if k + 3 < BPC:
                    load_xb(k + 3)
                    load_x2(k + 3)
                if k + 2 < BPC:
                    emit_A(k + 2)
                    emit_S(k + 2)
                emit_PZ(k)
                emit_T(k)
                if k == 3:
                    emit_O(0)
                if k >= 4:
                    emit_X(k - 4)
            emit_O(1)
            for b in range(4, BPC):
                emit_X(b)

            # ---- proj for all 8 batches ----
            cls_sb = const.tile([BPC, C], F32, tag="cls_sb")
            for ho, hw in C_HALVES:
                pc = ps.tile([BPC, 512], F32, tag="pav")
                for ci in range(CT):
                    nc.tensor.matmul(
                        pc[:, :hw], oT_vec[:, ci, :], wp[:, ci, ho:ho + hw],
                        start=(ci == 0), stop=(ci == CT - 1))
                nc.vector.tensor_add(
                    cls_sb[:, ho:ho + hw], pc[:, :hw], pbr[:, ho:ho + hw])
            nc.sync.dma_start(cls_d.ap(), cls_sb[:])

    nc.compile()
    return nc


@functools.lru_cache(maxsize=1)
def _module():
    return build_module()


def make_in_maps(x, qkv_w, qkv_b, proj_w, proj_b):
    x = np.asarray(x, dtype=np.float32)
    qkv_w = np.asarray(qkv_w, dtype=np.float32)
    qkv_b = np.asarray(qkv_b, dtype=np.float32)
    proj_w = np.asarray(proj_w, dtype=np.float32)
    proj_b = np.asarray(proj_b, dtype=np.float32)

    wk2 = np.ascontiguousarray(qkv_w[C:2 * C]).astype(NPBF16)       # [o, c]
    wv = np.ascontiguousarray(qkv_w[2 * C:].T).astype(NPBF16)       # [c, o]
    wq = np.ascontiguousarray(qkv_w[:C].T * SCALE).astype(NPBF16)   # [c, o]
    wp = np.ascontiguousarray(proj_w.T).astype(NPBF16)              # [c, o]
    qb = np.tile(qkv_b[:C] * SCALE, (BPC, 1)).astype(np.float32)    # [8, C]
    # v bias contributes exactly (vb @ proj_w.T) to cls; fold into proj bias
    pb_eff = proj_b + qkv_b[2 * C:] @ proj_w.T
    pb = np.tile(pb_eff, (BPC, 1)).astype(np.float32)               # [8, C]

    in_maps = []
    for i in range(NCORES):
        xs = x[i * BPC:(i + 1) * BPC]                               # [8, N, C]
        x2 = xs.astype(NPBF16)                                      # [8, N, C]
        xT = np.ascontiguousarray(xs.transpose(2, 0, 1)).astype(NPBF16)
        xcls = np.ascontiguousarray(xs[:, 0, :].T).astype(NPBF16)   # [C, 8]
        in_maps.append({
            "xT": xT, "x2": x2, "wk2": wk2, "wv": wv, "wq": wq, "wp": wp,
            "xcls": xcls, "qb": qb, "pb": pb,
        })
    return in_maps


def kernel(x, qkv_w, qkv_b, proj_w, proj_b):
    nc = _module()
    in_maps = make_in_maps(x, qkv_w, qkv_b, proj_w, proj_b)
    res = bass_utils.run_bass_kernel_spmd(
        nc, in_maps, core_ids=list(range(NCORES)))
    out = np.array(np.asarray(x), dtype=np.float32, copy=True)
    for i in range(NCORES):
        out[i * BPC:(i + 1) * BPC, 0, :] = res.results[i]["cls"]
    return out


# revision 15
# speedup vs baseline: 2.3650x; 1.1018x over previous
"""ClassAttention kernel for 8x TRN2 NeuronCores.

Reference computation (per batch element):
    qkv = x @ qkv_w.T + qkv_b                      # [N, 3C]
    q, k, v = split(qkv)                           # heads H=12, D=64
    s = softmax((q_cls . k) / sqrt(D))             # class-token query only
    cls = (s @ v) @ proj_w.T + proj_b              # [1, C]
    out = concat([cls, x[1:]])                     # rows 1..N pass through

Only the class token row changes, so the device kernel computes just the
[B, C] cls output; rows 1..N are passed through on the host.

Sharding: data-parallel over batch, 8 batches per core, no collectives.
Compute dtype bf16 (fp32 PSUM accumulation), fp32 output.

Key algebraic restructuring (exploits the single class-token query):
  - scores fold the k-projection into a per-batch effective query in
    x-space:  s[b,h,n] = sum_c Wt[c, b*12+h] x[b,n,c]  with
    Wt = wk.T @ blockdiag(q) computed ONCE for all batches (768x96).
    No k vector is ever materialized.
  - the v-projection commutes with the attention average:
    o = p.T @ (x @ wv.T) = (p.T @ x) @ wv.T, so we compute the
    attention-weighted average of x (Z = p.T @ x, 12x768 per batch)
    and project it through wv once per batch. No v vector is ever
    materialized. This collapses the dominant [577x768]x[768x768]
    per-batch projection into [12x768]x[768x768].
  - k bias shifts every score of a head by the same constant ->
    cancels in softmax; dropped.
  - v bias contributes sum_n(p)=1 times vb to the attention output ->
    folds into the proj bias on the host: pb_eff = proj_b + vb @ proj_w.T.
  - softmax normalization (1/sum) is applied while evacuating the
    attention-output psum.

Per-core dataflow (b = 0..8 batches, C=768 in 6 chunks of 128):
  xT[c, b, n], x2[b, n, c]   both layouts of x, host-cast bf16
  qc[b, o]    = wqT.T @ xcls + qb    (wq,qb pre-scaled by 1/8 on host)
  Qblk[o, bh] = block-diagonal q     (PE transpose + aligned copies)
  Wt[c, bh]   = wk2.T @ Qblk         (36 matmuls, once)
  s_b[h, n]   = Wt_b.T @ xT_b
  p_b         = exp(s - max)          (unnormalized, bf16)
  Z_b[h, c]   = pT_b.T @ x2_b
  o_b[h, o]   = ZT_b.T @ wvT          (12x768; diag blocks are the result)
  oT_vec[o,b] = r * diag-extract      (PE transpose + aligned copies)
  cls[b, :]   = oT.T @ wpT + pb_eff

Scores for batch b+1 are emitted ahead of batch b's softmax-dependent
work so the PE never waits on the DVE/ACT softmax chain.
"""

import functools

import numpy as np
import ml_dtypes

import concourse.bass as bass
import concourse.tile as tile
from concourse import bacc, mybir, masks
from concourse import bass_utils

BF16 = mybir.dt.bfloat16
F32 = mybir.dt.float32
NPBF16 = ml_dtypes.bfloat16

B, N, C = 64, 577, 768
H, D = 12, 64
NCORES = 8
BPC = B // NCORES          # 8 batches per core
CT = C // 128              # 6 chunks of the feature dim
SCALE = D ** -0.5          # folded into wq/qb on the host

# token splits: matmul free dim (<=512 fp32 psum bank), K-chunks (<=128)
N_HALVES = [(0, 289), (289, 288)]
C_HALVES = [(0, 512), (512, 256)]
T_TILES = [(0, 128), (128, 128), (256, 128), (384, 128), (512, 65)]
NTT = len(T_TILES)


def build_module():
    nc = bacc.Bacc("TRN2", target_bir_lowering=False, debug=False)

    xT_d = nc.dram_tensor("xT", [C, BPC, N], BF16, kind="ExternalInput")
    x2_d = nc.dram_tensor("x2", [BPC, N, C], BF16, kind="ExternalInput")
    wk2_d = nc.dram_tensor("wk2", [C, C], BF16, kind="ExternalInput")  # [o, c]
    wv_d = nc.dram_tensor("wv", [C, C], BF16, kind="ExternalInput")    # [c, o]
    wq_d = nc.dram_tensor("wq", [C, C], BF16, kind="ExternalInput")    # [c, o]
    wp_d = nc.dram_tensor("wp", [C, C], BF16, kind="ExternalInput")    # [c, o]
    xcls_d = nc.dram_tensor("xcls", [C, BPC], BF16, kind="ExternalInput")
    qb_d = nc.dram_tensor("qb", [BPC, C], F32, kind="ExternalInput")
    pb_d = nc.dram_tensor("pb", [BPC, C], F32, kind="ExternalInput")
    cls_d = nc.dram_tensor("cls", [BPC, C], F32, kind="ExternalOutput")

    AF = mybir.ActivationFunctionType
    AX = mybir.AxisListType

    with tile.TileContext(nc) as tc:
        with (
            tc.tile_pool(name="const", bufs=1) as const,
            tc.tile_pool(name="xp", bufs=4) as xp,
            tc.tile_pool(name="x2p", bufs=5) as x2p,
            tc.tile_pool(name="sm", bufs=3) as sm,
            tc.tile_pool(name="ps", bufs=2, space="PSUM") as ps,
        ):
            # ---- DMAs, in the order the pipeline consumes them ----
            # (the cost model serializes dma_starts on one channel)
            xcls = const.tile([128, CT, BPC], BF16, tag="xcls")
            nc.sync.dma_start(
                xcls[:], xcls_d.ap().rearrange("(a p) b -> p a b", p=128))
            wq = const.tile([128, CT, C], BF16, tag="wq")
            wqr = wq_d.ap().rearrange("(a p) o -> p a o", p=128)
            for ci in range(CT):
                nc.sync.dma_start(wq[:, ci, :], wqr[:, ci, :])
            qbr = const.tile([BPC, C], F32, tag="qbr")
            nc.sync.dma_start(qbr[:], qb_d.ap())
            wk2 = const.tile([128, CT, C], BF16, tag="wk2")
            wk2r = wk2_d.ap().rearrange("(a p) o -> p a o", p=128)
            for ci in range(CT):
                nc.sync.dma_start(wk2[:, ci, :], wk2r[:, ci, :])

            xbs = {}

            def load_xb(b):
                xb = xp.tile([128, CT, N], BF16, tag="xb")
                nc.sync.dma_start(
                    xb[:],
                    xT_d.ap()[:, b, :].rearrange("(a p) t -> p a t", p=128))
                xbs[b] = xb

            x2s = {}

            def load_x2(b):
                x2 = x2p.tile([128, NTT, C], BF16, tag="x2")
                for ti, (to, tw) in enumerate(T_TILES):
                    nc.sync.dma_start(
                        x2[:tw, ti, :], x2_d.ap()[b, to:to + tw, :])
                x2s[b] = x2

            load_xb(0)
            load_x2(0)
            wv = const.tile([128, CT, C], BF16, tag="wv")
            wvr = wv_d.ap().rearrange("(a p) o -> p a o", p=128)
            nc.sync.dma_start(wv[:, :, 0:512], wvr[:, :, 0:512])
            load_xb(1)
            nc.sync.dma_start(wv[:, :, 512:768], wvr[:, :, 512:768])
            load_x2(1)
            load_xb(2)
            load_x2(2)
            pbr = const.tile([BPC, C], F32, tag="pbr")
            wp = const.tile([128, CT, C], BF16, tag="wp")

            identb = const.tile([12, 12], BF16, tag="identb")
            masks.make_identity(nc, identb[:])
            identf = const.tile([BPC, BPC], F32, tag="identf")
            masks.make_identity(nc, identf[:])

            Qblk = const.tile([128, CT, BPC * H], BF16, tag="Qblk")
            nc.vector.memset(Qblk[:], 0.0)
            Wt = const.tile([128, CT, BPC * H], BF16, tag="Wt")
            oT_vec = const.tile([128, CT, BPC], BF16, tag="oT_vec")
            q_sb = const.tile([BPC, C], F32, tag="q_sb")
            # ZT for 4 batches per 128-col group, 32-aligned (zero padding)
            ZT_all = const.tile([128, CT, 2, 128], BF16, tag="ZT_all")
            nc.vector.memset(ZT_all[:], 0.0)
            o_sb = const.tile([128, 2, C], BF16, tag="o_sb")
            # identity replicated at partition bases 0/32/64/96 for the
            # 32-aligned diag-extraction transposes
            ident4 = const.tile([128, H], BF16, tag="ident4")
            for _j in range(4):
                masks.make_identity(nc, ident4[32 * _j:32 * _j + H, :])

            # ---- q for all 8 batches, Qblk, Wt ----
            for ho, hw in C_HALVES:
                pq = ps.tile([BPC, 512], F32, tag="pav")
                for ci in range(CT):
                    nc.tensor.matmul(
                        pq[:, :hw], xcls[:, ci, :], wq[:, ci, ho:ho + hw],
                        start=(ci == 0), stop=(ci == CT - 1))
                nc.vector.tensor_add(
                    q_sb[:, ho:ho + hw], pq[:, :hw], qbr[:, ho:ho + hw])

            QblkV = Qblk[:].rearrange("p a (b h) -> p a b h", h=H)
            for ci in range(CT):
                pqt = ps.tile([128, BPC], F32, tag="pav")
                nc.tensor.transpose(
                    pqt[:, :], q_sb[:, ci * 128:(ci + 1) * 128],
                    identf[:BPC, :BPC])
                for j in range(2):
                    h = 2 * ci + j
                    nc.vector.tensor_copy(
                        QblkV[j * 64:(j + 1) * 64, ci, :, h],
                        pqt[j * 64:(j + 1) * 64, :])

            for cj in range(CT):
                pw = ps.tile([128, BPC * H], F32, tag="pav")
                for oj in range(CT):
                    nc.tensor.matmul(
                        pw[:, :], wk2[:, oj, cj * 128:(cj + 1) * 128],
                        Qblk[:, oj, :],
                        start=(oj == 0), stop=(oj == CT - 1))
                nc.vector.tensor_copy(Wt[:, cj, :], pw[:, :])

            # ---- per-batch software pipeline ----
            # Emission (= PE execution) order per slot k:
            #   A(k+2) scores | S(k+2) softmax (DVE/ACT) | X(k-1) extract |
            #   P(k) pT | Z(k) | T(k) ZT | O(k) attn-out
            # so every DVE/ACT round-trip is covered by PE work from a
            # neighboring slot. Psum evacuations are interleaved inside the
            # Z/O chains (half0 evacuates while half1's matmuls stream).
            st = {}

            def emit_A(b):
                xb = xbs.pop(b)
                pss = []
                for no, nw in N_HALVES:
                    s_ps = ps.tile([H, 512], F32, tag="pscz")
                    for ci in range(CT):
                        nc.tensor.matmul(
                            s_ps[:, :nw],
                            Wt[:, ci, b * H:(b + 1) * H],
                            xb[:, ci, no:no + nw],
                            start=(ci == 0), stop=(ci == CT - 1))
                    pss.append(s_ps)
                st[b] = {"pss": pss}

            def emit_S(b):
                s = st[b]
                pss = s["pss"]
                m1 = sm.tile([H, 1], F32, tag="m1")
                m2 = sm.tile([H, 1], F32, tag="m2")
                nc.vector.reduce_max(m1[:], pss[0][:, :N_HALVES[0][1]],
                                     axis=AX.X)
                nc.vector.reduce_max(m2[:], pss[1][:, :N_HALVES[1][1]],
                                     axis=AX.X)
                am = sm.tile([H, 1], F32, tag="am")
                nc.vector.tensor_max(am[:], m1[:], m2[:])
                negm = sm.tile([H, 1], F32, tag="negm")
                nc.scalar.mul(negm[:], am[:], -1.0)

                e_bf = sm.tile([H, N], BF16, tag="e_bf")
                sums = []
                for i, (no, nw) in enumerate(N_HALVES):
                    acc = sm.tile([H, 1], F32, tag=f"acc{i}")
                    nc.scalar.activation(
                        e_bf[:, no:no + nw], pss[i][:, :nw], AF.Exp,
                        bias=negm[:], scale=1.0, accum_out=acc[:])
                    sums.append(acc)
                ssum = sm.tile([H, 1], F32, tag="ssum")
                nc.vector.tensor_add(ssum[:], sums[0][:], sums[1][:])
                rden = sm.tile([H, 1], F32, tag="rden")
                nc.vector.reciprocal(rden[:], ssum[:])
                # normalized attention weights p = e / sum (bf16)
                e_nm = sm.tile([H, N], BF16, tag="e_nm")
                nc.vector.tensor_scalar_mul(e_nm[:], e_bf[:], rden[:])
                s["e_nm"] = e_nm

            def emit_PZ(b):
                s = st[b]
                e_nm = s["e_nm"]
                pT = sm.tile([128, NTT, H], BF16, tag="pT")
                for ti, (to, tw) in enumerate(T_TILES):
                    ppt = ps.tile([128, H], BF16, tag="ptrb")
                    nc.tensor.transpose(
                        ppt[:tw, :], e_nm[:, to:to + tw], identb[:H, :H])
                    nc.vector.tensor_copy(pT[:tw, ti, :], ppt[:tw, :])

                x2 = x2s.pop(b)
                z_sb = sm.tile([H, C], BF16, tag="z_sb")
                for ho, hw in C_HALVES:
                    pz = ps.tile([H, 512], F32, tag="pscz")
                    for ti, (to, tw) in enumerate(T_TILES):
                        nc.tensor.matmul(
                            pz[:, :hw],
                            pT[:tw, ti, :],
                            x2[:tw, ti, ho:ho + hw],
                            start=(ti == 0), stop=(ti == NTT - 1))
                    nc.scalar.copy(z_sb[:, ho:ho + hw], pz[:, :hw])
                s["z_sb"] = z_sb

            def emit_T(b):
                # ZT for batch b into its 32-aligned column group of ZT_all
                s = st.pop(b)
                z_sb = s["z_sb"]
                g, j = b // 4, b % 4
                for ci in range(CT):
                    pzt = ps.tile([128, H], BF16, tag="ptrb")
                    nc.tensor.transpose(
                        pzt[:, :], z_sb[:, ci * 128:(ci + 1) * 128],
                        identb[:H, :H])
                    nc.vector.tensor_copy(
                        ZT_all[:, ci, g, 32 * j:32 * j + H], pzt[:, :])

            def emit_O(g):
                # o rows for 4 batches at once: psum rows 32j..32j+12 = batch
                # 4g+j  (M-packed; zero columns of ZT_all give zero rows)
                for ho, hw in C_HALVES:
                    po = ps.tile([128, 512], F32, tag="pav")
                    for ci in range(CT):
                        nc.tensor.matmul(
                            po[:, :hw],
                            ZT_all[:, ci, g, :],
                            wv[:, ci, ho:ho + hw],
                            start=(ci == 0), stop=(ci == CT - 1))
                    nc.scalar.copy(o_sb[:, g, ho:ho + hw], po[:, :hw])

            def emit_X(b):
                # extract diagonal blocks of batch b, transposed: oT_vec[o, b]
                g, j = b // 4, b % 4
                for ci in range(CT):
                    pot = ps.tile([128, H], BF16, tag="ptrb")
                    nc.tensor.transpose(
                        pot[:, :],
                        o_sb[32 * j:32 * j + H, g, ci * 128:(ci + 1) * 128],
                        ident4[32 * j:32 * j + H, :],
                        tile_position=(32 * j, 0))
                    for jj in range(2):
                        h = 2 * ci + jj
                        nc.vector.tensor_copy(
                            oT_vec[jj * 64:(jj + 1) * 64, ci, b:b + 1],
                            pot[jj * 64:(jj + 1) * 64, h:h + 1])

            emit_A(0)
            emit_S(0)
            emit_A(1)
            emit_S(1)
            for k in range(BPC):
                if k + 3 < BPC:
                    load_xb(k + 3)
                    load_x2(k + 3)
                if k == 5:
                    nc.sync.dma_start(pbr[:], pb_d.ap())
                    nc.sync.dma_start(
                        wp[:], wp_d.ap().rearrange("(a p) o -> p a o", p=128))
                if k + 2 < BPC:
                    emit_A(k + 2)
                    emit_S(k + 2)
                emit_PZ(k)
                emit_T(k)
                if k == 3:
                    emit_O(0)
                if k >= 4:
                    emit_X(k - 4)
            emit_O(1)
            for b in range(4, BPC):
                emit_X(b)

            # ---- proj for all 8 batches ----
            cls_sb = const.tile([BPC, C], F32, tag="cls_sb")
            for ho, hw in C_HALVES:
                pc = ps.tile([BPC, 512], F32, tag="pav")
                for ci in range(CT):
                    nc.tensor.matmul(
                        pc[:, :hw], oT_vec[:, ci, :], wp[:, ci, ho:ho + hw],
                        start=(ci == 0), stop=(ci == CT - 1))
                nc.vector.tensor_add(
                    cls_sb[:, ho:ho + hw], pc[:, :hw], pbr[:, ho:ho + hw])
            nc.sync.dma_start(cls_d.ap(), cls_sb[:])

    nc.compile()
    return nc


@functools.lru_cache(maxsize=1)
def _module():
    return build_module()


def make_in_maps(x, qkv_w, qkv_b, proj_w, proj_b):
    x = np.asarray(x, dtype=np.float32)
    qkv_w = np.asarray(qkv_w, dtype=np.float32)
    qkv_b = np.asarray(qkv_b, dtype=np.float32)
    proj_w = np.asarray(proj_w, dtype=np.float32)
    proj_b = np.asarray(proj_b, dtype=np.float32)

    wk2 = np.ascontiguousarray(qkv_w[C:2 * C]).astype(NPBF16)       # [o, c]
    wv = np.ascontiguousarray(qkv_w[2 * C:].T).astype(NPBF16)       # [c, o]
    wq = np.ascontiguousarray(qkv_w[:C].T * SCALE).astype(NPBF16)   # [c, o]
    wp = np.ascontiguousarray(proj_w.T).astype(NPBF16)              # [c, o]
    qb = np.tile(qkv_b[:C] * SCALE, (BPC, 1)).astype(np.float32)    # [8, C]
    # v bias contributes exactly (vb @ proj_w.T) to cls; fold into proj bias
    pb_eff = proj_b + qkv_b[2 * C:] @ proj_w.T
    pb = np.tile(pb_eff, (BPC, 1)).astype(np.float32)               # [8, C]

    in_maps = []
    for i in range(NCORES):
        xs = x[i * BPC:(i + 1) * BPC]                               # [8, N, C]
        x2 = xs.astype(NPBF16)                                      # [8, N, C]
        xT = np.ascontiguousarray(xs.transpose(2, 0, 1)).astype(NPBF16)
        xcls = np.ascontiguousarray(xs[:, 0, :].T).astype(NPBF16)   # [C, 8]
        in_maps.append({
            "xT": xT, "x2": x2, "wk2": wk2, "wv": wv, "wq": wq, "wp": wp,
            "xcls": xcls, "qb": qb, "pb": pb,
        })
    return in_maps


def kernel(x, qkv_w, qkv_b, proj_w, proj_b):
    nc = _module()
    in_maps = make_in_maps(x, qkv_w, qkv_b, proj_w, proj_b)
    res = bass_utils.run_bass_kernel_spmd(
        nc, in_maps, core_ids=list(range(NCORES)))
    out = np.array(np.asarray(x), dtype=np.float32, copy=True)
    for i in range(NCORES):
        out[i * BPC:(i + 1) * BPC, 0, :] = res.results[i]["cls"]
    return out
